# revision 40
# baseline (speedup 1.0000x reference)
"""Trainium2 Bass kernel for a 3-layer GCN encoder with global max pool.

Strategy (8 NeuronCores, SPMD, 5 launches, host staging between launches):
  - Nodes partitioned graph-wise (graph g -> core g//64). The host only MOVES
    device-computed bytes between launches (gather rows into padded message
    tables); every FLOP runs on device.
  - Aggregation layers stage per-edge messages in fp8 (e3m4) with fixed
    power-free scale factors folded into the device-side s-tables, halving
    HBM traffic vs fp16.
  - The aggregation sum runs on the Tensor engine: identity-weight matmuls
    accumulate message strips into PSUM (start/stop prefix accumulation).
    Columns (node pairs) are sorted by descending in-degree so the set of
    columns with a message at depth d is a prefix; strips are stored
    region-major (one 512-column PSUM bank region at a time) so PSUM holds
    each region until its sum completes.
  - Post-ops per region: DVE multiplies PSUM by the s-table; Activation
    applies (scaled) bias+relu; DVE writes the next layer's pre-scaled
    fp8 table directly.
  - Launches: L1  T1 = (s*M1) * (X @ W1)                 [fp8 out]
              L2  T2 = (s*M2) * relu(s*Agg(T1)/M1 + b1)  [fp8 out]
              L3  T3 = (s*M3) * (relu((s*Agg(T2)/M2)@W2 + b2)@W3)
              L4  H3 = s*Agg(T3)/M3 + b3                 [fp16 out]
              L5  per-graph max pool over H3 (depth-major staged layout)
"""

import numpy as np
import ml_dtypes

N = 50000
IN_DIM = 128
HID = 64
F2 = 2 * HID
N_GRAPHS = 512
C = 8
P = 128
GPC = N_GRAPHS // C
RW = 512            # psum region width (columns)
F16 = np.float16
F8 = ml_dtypes.float8_e3m4

M1, M2, M3 = 5.0, 11.0, 44.0   # staging scale factors (fold into s-tables)

# aggregation offload: region -> (PE strip count, engine for the rest)
_L3_OFF = {1: (10, "dve"), 2: (12, "pool"), 3: (10, "dve")}
_AGG_OFF = {0: (16, "dve"), 1: (11, "dve"), 2: (11, "pool"), 3: (10, "dve")}


# --------------------------------------------------------------------------
# Host-side preprocessing (graph structure only - no feature arithmetic)
# --------------------------------------------------------------------------

def _host_prep(edge_index, batch):
    src = np.asarray(edge_index[0], dtype=np.int64)
    dst = np.asarray(edge_index[1], dtype=np.int64)
    batch = np.asarray(batch, dtype=np.int64)
    core_of = batch // GPC

    indeg = np.bincount(dst, minlength=N)
    k = indeg + 1                     # slots per node incl. self loop
    s = (1.0 / np.sqrt(k.astype(np.float64))).astype(np.float32)

    # in-neighbor lists grouped by dst
    eorder = np.argsort(dst, kind="stable")
    esrc = src[eorder]
    estart = np.zeros(N + 1, np.int64)
    np.cumsum(np.bincount(dst, minlength=N), out=estart[1:])

    # per-core node order: descending k, paired (2i, 2i+1) into columns
    orders = []
    for c in range(C):
        nodes = np.nonzero(core_of == c)[0]
        orders.append(nodes[np.argsort(-k[nodes], kind="stable")])
    NCOL = max((len(o) + 1) // 2 for o in orders)

    tops = np.full((C, NCOL), -1, np.int64)
    bots = np.full((C, NCOL), -1, np.int64)
    for c in range(C):
        o = orders[c]
        tops[c, : len(o[0::2])] = o[0::2]
        bots[c, : len(o[1::2])] = o[1::2]
    topsx = np.where(tops >= 0, tops, N)
    botsx = np.where(bots >= 0, bots, N)

    # column depth = max over cores of max(k_top, k_bot); >=1 (self loop)
    kk = np.concatenate([k, [0]])
    D_col = np.maximum(kk[topsx], kk[botsx]).max(axis=0)
    D_col = np.maximum(D_col, 1)
    assert (np.diff(D_col) <= 0).all()
    DMAX = int(D_col[0])
    n_d = np.array([(D_col > d).sum() for d in range(DMAX)], np.int64)

    # region widths: full PSUM banks, with the remainder split into a
    # shrinking taper so the final post-op chains are short
    widths = []
    rem = NCOL
    while rem > 704:
        widths.append(RW)
        rem -= RW
    if rem > 384:
        w1 = (rem * 33 // 64) & ~31
        w2 = ((rem - w1) * 3 // 5) & ~31
        widths += [w1, w2, rem - w1 - w2]
    elif rem > 160:
        w1 = (rem * 3 // 5) & ~31
        widths += [w1, rem - w1]
    else:
        widths.append(rem)

    # region-major strips: (region_col0, d, w, slot_off)
    strips = []
    off = 0
    regions = []          # (col0, width, D_r, [strip indices])
    r0 = 0
    for wr in widths:
        Dr = int(D_col[r0])
        idxs = []
        for d in range(Dr):
            w = int(min(n_d[d] - r0, wr))
            assert w > 0
            idxs.append(len(strips))
            strips.append((r0, d, w, off))
            off += w
        regions.append((r0, wr, Dr, idxs))
        r0 += wr
    SLOTS = off

    # slot -> source node maps (N = zero row) for tops/bottoms
    indegx = np.concatenate([indeg, [0]])
    estartx = np.concatenate([estart[:-1], [0]])
    srcmap = np.full((C, 2, SLOTS), N, np.int64)
    for (r0, d, w, soff) in strips:
        for c in range(C):
            for half, nodes_h in ((0, topsx[c]), (1, botsx[c])):
                v = nodes_h[r0 : r0 + w]
                if d == 0:
                    srcmap[c, half, soff : soff + w] = v
                else:
                    sel = (d <= indegx[v]) & (v < N)
                    tgt = srcmap[c, half, soff : soff + w]
                    tgt[sel] = esrc[estartx[v[sel]] + d - 1]

    # s rows [C, 2, NCOL] fp16 (top/bot); the launch-specific scale constant
    # rides in the tiny broadcast matrix E2 instead of a full table.
    sx = np.concatenate([s, [0.0]]).astype(F16)
    SROW = np.stack([sx[topsx], sx[botsx]], axis=1)    # [C, 2, NCOL]

    # L3 keeps a full [128, NCOL] table (PE there is the bottleneck)
    sx3 = np.concatenate([s * (1.0 / M2), [0.0]]).astype(F16)
    top = sx3[topsx][:, None, :].repeat(HID, axis=1)
    bot = sx3[botsx][:, None, :].repeat(HID, axis=1)
    SA3 = np.concatenate([top, bot], axis=1)           # [C, 128, NCOL]

    # pooling: graphs ranked by size per core, split into NPG groups of GH;
    # group G is depth-major: column offG + d*GH + j = d-th node pair of the
    # (G*GH+j)-th largest graph.  Per-group depth S2G trims the rectangle.
    NPG = 4
    GH = GPC // NPG
    gl = batch % GPC
    cnt = np.zeros((C, GPC), np.int64)
    np.add.at(cnt, (core_of, gl), 1)
    grank = np.argsort(-cnt, axis=1, kind="stable")     # [C, GPC] rank->graph
    pairs = -(-cnt // 2)
    S2G = []
    for G in range(NPG):
        S2G.append(int(max(pairs[c, grank[c, G * GH]] for c in range(C))))
    offG = np.zeros(NPG + 1, np.int64)
    np.cumsum(np.array(S2G) * GH, out=offG[1:])
    POOLW = int(offG[-1])
    poolmap = np.full((C, 2, POOLW), N, np.int64)
    for c in range(C):
        for j in range(GPC):
            g = grank[c, j]
            nodes = np.nonzero((core_of == c) & (gl == g))[0]
            e = nodes[0::2]
            o = nodes[1::2]
            G = j // GH
            base = int(offG[G]) + (j % GH)
            poolmap[c, 0, base : base + S2G[G] * GH : GH][: len(e)] = e
            poolmap[c, 1, base : base + S2G[G] * GH : GH][: len(o)] = o

    meta = dict(NCOL=NCOL, SLOTS=SLOTS, strips=strips, regions=regions,
                NPG=NPG, GH=GH, S2G=S2G, offG=[int(x) for x in offG],
                POOLW=POOLW)
    return dict(meta=meta, tops=tops, bots=bots, topsx=topsx, botsx=botsx,
                srcmap=srcmap, poolmap=poolmap, cnt=cnt, grank=grank,
                SROW=SROW, SA3=SA3)


# --------------------------------------------------------------------------
# Bass programs
# --------------------------------------------------------------------------

def _mk_bass():
    import concourse.bacc as bacc
    return bacc.Bacc(None)


def _msg_chunks(strips, SLOTS, first=1400, later=3600):
    """Split the slot axis into DMA chunks at strip boundaries."""
    cuts = []
    target = first
    for (r0, d, w, soff) in strips:
        end = soff + w
        if end >= target:
            cuts.append(end)
            target = end + later
    if not cuts or cuts[-1] != SLOTS:
        cuts.append(SLOTS)
    out = []
    a = 0
    for b in cuts:
        out.append((a, b))
        a = b
    return out


def _prog_agg(meta, layer):
    """L2 (layer==2): OUT = (s*M2)*relu((M1*M2)*(A*s/M1) + (M1*M2)*b1), fp8
       L4 (layer==4): OUT = A*s/M3 + b3, fp16"""
    import concourse.mybir as mybir
    import concourse.tile as tile
    f8 = mybir.dt.float8e3
    f16 = mybir.dt.float16
    f32 = mybir.dt.float32
    Alu = mybir.AluOpType
    Act = mybir.ActivationFunctionType
    NCOL, SLOTS = meta["NCOL"], meta["SLOTS"]
    nc = _mk_bass()

    MSG_d = nc.dram_tensor("MSG", [P, SLOTS], f8, kind="ExternalInput")
    SROW_d = nc.dram_tensor("SROW", [2, NCOL], f16, kind="ExternalInput")
    E2_d = nc.dram_tensor("E2", [2, P], f16, kind="ExternalInput")
    BK_d = nc.dram_tensor("BK", [P, 1], f32, kind="ExternalInput")
    ID_d = nc.dram_tensor("ID", [P, P], f8, kind="ExternalInput")
    OUT_d = nc.dram_tensor("OUT", [P, NCOL], f8 if layer == 2 else f16,
                           kind="ExternalOutput")

    OFF = dict(_AGG_OFF)

    with tile.TileContext(nc, num_cores=C) as tc:
        with (
            tc.tile_pool(name="const", bufs=1) as const,
            tc.tile_pool(name="ps", bufs=3, space="PSUM") as psp,
        ):
            ID_s = const.tile([P, P], f8)
            nc.sync.dma_start(ID_s[:], ID_d[:])
            SROW_s = const.tile([2, NCOL], f16)
            nc.sync.dma_start(SROW_s[:], SROW_d[:])
            E2_s = const.tile([2, P], f16)
            nc.sync.dma_start(E2_s[:], E2_d[:])
            BK_s = const.tile([P, 1], f32)
            nc.sync.dma_start(BK_s[:], BK_d[:])
            MSG_s = const.tile([P, SLOTS], f8)
            chunks = _msg_chunks(meta["strips"], SLOTS)
            for (a, b) in chunks:
                nc.sync.dma_start(MSG_s[:, a:b], MSG_d[:, a:b])
            SA_s = const.tile([P, NCOL], f16)
            U_s = const.tile([P, NCOL], f16)
            H_s = const.tile([P, NCOL], f16)
            OUT_s = const.tile([P, NCOL], f8 if layer == 2 else f16)
            ACC = const.tile([P, NCOL], f16)

            # PE warm-up during the first MSG DMA (pstate ramp), then
            # broadcast the s-row to the [128, NCOL] scale table via PE;
            # Act moves it to SBUF
            wm = psp.tile([P, RW], f32, tag="ps")
            for i in range(6):
                nc.tensor.matmul(wm[:, :P], lhsT=ID_s[:], rhs=ID_s[:],
                                 start=(i == 0), stop=(i == 5))
            for a in range(0, NCOL, RW):
                w = min(RW, NCOL - a)
                psSA = psp.tile([P, RW], f32, tag="ps")
                nc.tensor.matmul(psSA[:, :w], lhsT=E2_s[:],
                                 rhs=SROW_s[:, a : a + w],
                                 start=True, stop=True)
                nc.scalar.activation(out=SA_s[:, a : a + w],
                                     in_=psSA[:, :w], func=Act.Copy,
                                     bias=0.0, scale=1.0)

            nreg = len(meta["regions"])
            stored = [0]

            def fin(ri):
                # final OUT mult + store, emitted one region late so the
                # engines pipeline across the last two regions
                (r0, wr, Dr, idxs) = meta["regions"][ri]
                cols = slice(r0, r0 + wr)
                if layer == 2:
                    eng = nc.gpsimd if ri in (1, 3) else nc.vector
                    eng.tensor_tensor(out=OUT_s[:, cols], in0=H_s[:, cols],
                                      in1=SA_s[:, cols], op=Alu.mult)
                if ri % 2 == 1 or ri >= nreg - 3 or r0 + wr == NCOL:
                    nc.sync.dma_start(OUT_d[:, stored[0] : r0 + wr],
                                      OUT_s[:, stored[0] : r0 + wr])
                    stored[0] = r0 + wr

            for ri, (r0, wr, Dr, idxs) in enumerate(meta["regions"]):
                split, eng_name = OFF.get(ri, (Dr, None))
                split = min(split, Dr)
                if split < Dr:
                    aeng = nc.vector if eng_name == "dve" else nc.gpsimd
                    first = True
                    for si in idxs[split:]:
                        (_, d, w, soff) = meta["strips"][si]
                        strip = MSG_s[:, soff : soff + w]
                        if first:
                            aeng.tensor_copy(ACC[:, r0 : r0 + w], strip)
                            first = False
                        else:
                            aeng.tensor_tensor(out=ACC[:, r0 : r0 + w],
                                               in0=ACC[:, r0 : r0 + w],
                                               in1=strip, op=Alu.add)
                ps = psp.tile([P, RW], f32, tag="ps")
                for j, si in enumerate(idxs[:split]):
                    (_, d, w, soff) = meta["strips"][si]
                    nc.tensor.matmul(ps[:, :w], lhsT=ID_s[:],
                                     rhs=MSG_s[:, soff : soff + w],
                                     start=(j == 0),
                                     stop=(j == Dr - 1 and split == Dr))
                if split < Dr:
                    (_, d, w, soff) = meta["strips"][idxs[split]]
                    nc.tensor.matmul(ps[:, :w], lhsT=ID_s[:],
                                     rhs=ACC[:, r0 : r0 + w],
                                     start=False, stop=True)
                cols = slice(r0, r0 + wr)
                nc.vector.tensor_tensor(out=U_s[:, cols], in0=ps[:, :wr],
                                        in1=SA_s[:, cols], op=Alu.mult)
                if layer == 2:
                    nc.scalar.activation(out=H_s[:, cols], in_=U_s[:, cols],
                                         func=Act.Relu, bias=BK_s[:],
                                         scale=float(M1 * M2))
                else:
                    nc.scalar.activation(out=OUT_s[:, cols], in_=U_s[:, cols],
                                         func=Act.Identity, bias=BK_s[:],
                                         scale=1.0)
                if ri >= 1:
                    fin(ri - 1)
            fin(nreg - 1)
    nc.compile()
    return nc


def _prog_l1(meta):
    """T1 = (s*M1) * (X @ W1), two [64, NCOL] fp8 half strips."""
    import concourse.mybir as mybir
    import concourse.tile as tile
    f8 = mybir.dt.float8e3
    f16 = mybir.dt.float16
    f32 = mybir.dt.float32
    Alu = mybir.AluOpType
    Act = mybir.ActivationFunctionType
    NCOL = meta["NCOL"]
    nc = _mk_bass()

    XT_d = nc.dram_tensor("XT", [IN_DIM, 2 * NCOL], f16, kind="ExternalInput")
    W1_d = nc.dram_tensor("W1", [IN_DIM, HID], f16, kind="ExternalInput")
    SROW_d = nc.dram_tensor("SROW", [2, NCOL], f16, kind="ExternalInput")
    E2_d = nc.dram_tensor("E2", [2, P], f16, kind="ExternalInput")
    T1P_d = nc.dram_tensor("T1P", [P, NCOL], f8, kind="ExternalOutput")

    with tile.TileContext(nc, num_cores=C) as tc:
        with (
            tc.tile_pool(name="const", bufs=1) as const,
            tc.tile_pool(name="ps", bufs=4, space="PSUM") as psp,
        ):
            W1_s = const.tile([IN_DIM, HID], f16)
            nc.sync.dma_start(W1_s[:], W1_d[:])
            SROW_s = const.tile([2, NCOL], f16)
            nc.sync.dma_start(SROW_s[:], SROW_d[:])
            E2_s = const.tile([2, P], f16)
            nc.sync.dma_start(E2_s[:], E2_d[:])
            XT_s = const.tile([IN_DIM, 2 * NCOL], f16)
            SRT1_s = const.tile([P, NCOL], f16)
            cuts = [RW] + list(range(2 * RW, NCOL, 2 * RW)) + [NCOL]
            chunks = []
            a = 0
            for b in cuts:
                if b > a:
                    chunks.append((a, b))
                    a = b
            for ci, (a, b) in enumerate(chunks):
                nc.sync.dma_start(XT_s[:, a:b], XT_d[:, a:b])
                nc.sync.dma_start(XT_s[:, NCOL + a : NCOL + b],
                                  XT_d[:, NCOL + a : NCOL + b])
            T1_s = const.tile([P, NCOL], f8)
            V_s = const.tile([P, NCOL], f16)

            # warm up PE, then broadcast the s-row into SRT1 via PE + Act
            wm = psp.tile([P, RW], f32, tag="ps")
            for i in range(4):
                nc.tensor.matmul(wm[0:HID, :HID], lhsT=W1_s[:], rhs=W1_s[:],
                                 start=(i == 0), stop=(i == 3))
            for a in range(0, NCOL, RW):
                w = min(RW, NCOL - a)
                psSA = psp.tile([P, RW], f32, tag="ps")
                nc.tensor.matmul(psSA[:, :w], lhsT=E2_s[:],
                                 rhs=SROW_s[:, a : a + w],
                                 start=True, stop=True)
                nc.scalar.activation(out=SRT1_s[:, a : a + w],
                                     in_=psSA[:, :w], func=Act.Copy,
                                     bias=0.0, scale=1.0)

            stored = 0
            nflows = -(-NCOL // RW)
            for i in range(nflows):
                a = i * RW
                w = min(RW, NCOL - a)
                ps = psp.tile([P, RW], f32, tag="ps")
                for half in range(2):
                    nc.tensor.matmul(
                        ps[half * HID : half * HID + HID, :w], lhsT=W1_s[:],
                        rhs=XT_s[:, half * NCOL + a : half * NCOL + a + w],
                        start=True, stop=True)
                if i in (2, 4):
                    # relieve DVE: Act copies PSUM out, Pool applies the scale
                    nc.scalar.activation(out=V_s[:, a : a + w],
                                         in_=ps[:, :w], func=Act.Copy,
                                         bias=0.0, scale=1.0)
                    nc.gpsimd.tensor_tensor(
                        out=T1_s[:, a : a + w], in0=V_s[:, a : a + w],
                        in1=SRT1_s[:, a : a + w], op=Alu.mult)
                else:
                    nc.vector.tensor_tensor(
                        out=T1_s[:, a : a + w], in0=ps[:, :w],
                        in1=SRT1_s[:, a : a + w], op=Alu.mult)
                if i % 2 == 1 or a + w == NCOL:
                    nc.scalar.dma_start(T1P_d[:, stored : a + w],
                                        T1_s[:, stored : a + w])
                    stored = a + w
    nc.compile()
    return nc


def _prog_l3(meta):
    """T3 = (s*M3)*(relu((s*Agg(T2)/M2)@W2 + b2)@W3), fp8 half strips.
    W3 input is pre-scaled by M2*M3 on the host."""
    import concourse.mybir as mybir
    import concourse.tile as tile
    f8 = mybir.dt.float8e3
    f16 = mybir.dt.float16
    f32 = mybir.dt.float32
    Alu = mybir.AluOpType
    Act = mybir.ActivationFunctionType
    NCOL, SLOTS = meta["NCOL"], meta["SLOTS"]
    nc = _mk_bass()

    MSG_d = nc.dram_tensor("MSG", [P, SLOTS], f8, kind="ExternalInput")
    SA_d = nc.dram_tensor("SA", [P, NCOL], f16, kind="ExternalInput")
    W2_d = nc.dram_tensor("W2", [P, F2], f16, kind="ExternalInput")
    W3_d = nc.dram_tensor("W3", [F2, HID], f16, kind="ExternalInput")
    B2_d = nc.dram_tensor("B2", [F2, 1], f32, kind="ExternalInput")
    ID_d = nc.dram_tensor("ID", [P, P], f8, kind="ExternalInput")
    T3P_d = nc.dram_tensor("T3P", [P, NCOL], f8, kind="ExternalOutput")

    # deep strips of these regions are tree-summed on DVE/Pool into an fp16
    # partial; PE folds the partial into PSUM with one extra pass
    OFF = dict(_L3_OFF)

    with tile.TileContext(nc, num_cores=C) as tc:
        with (
            tc.tile_pool(name="const", bufs=1) as const,
            tc.tile_pool(name="psA", bufs=2, space="PSUM") as psAp,
            tc.tile_pool(name="ps2", bufs=4, space="PSUM") as ps2p,
            tc.tile_pool(name="ps3", bufs=2, space="PSUM") as ps3p,
        ):
            ID_s = const.tile([P, P], f8)
            nc.sync.dma_start(ID_s[:], ID_d[:])
            MSG_s = const.tile([P, SLOTS], f8)
            chunks = _msg_chunks(meta["strips"], SLOTS)
            SA_s = const.tile([P, NCOL], f16)
            W2_s = const.tile([P, F2], f16)
            W3_s = const.tile([F2, HID], f16)
            B2_s = const.tile([F2, 1], f32)
            for ci, (a, b) in enumerate(chunks):
                nc.sync.dma_start(MSG_s[:, a:b], MSG_d[:, a:b])
                if ci == 0:
                    nc.sync.dma_start(W2_s[:], W2_d[:])
                    nc.sync.dma_start(W3_s[:], W3_d[:])
                    nc.sync.dma_start(B2_s[:], B2_d[:])
                elif ci == 2:
                    nc.sync.dma_start(SA_s[:], SA_d[:])
            U_s = const.tile([P, NCOL], f16)
            H2E = const.tile([F2, NCOL], f16)
            H2O = const.tile([F2, NCOL], f16)
            T3_s = const.tile([P, NCOL], f8)
            ACC = const.tile([P, NCOL], f16)

            # PE warm-up during the first MSG DMA
            wm = psAp.tile([P, RW], f32, tag="psA")
            for i in range(8):
                nc.tensor.matmul(wm[:, :P], lhsT=ID_s[:], rhs=ID_s[:],
                                 start=(i == 0), stop=(i == 7))

            stored = [0]
            nreg = len(meta["regions"])

            def stage_a(ri):
                (r0, wr, Dr, idxs) = meta["regions"][ri]
                ps = ps_of[ri]
                cols = slice(r0, r0 + wr)
                nc.vector.tensor_tensor(out=U_s[:, cols], in0=ps[:, :wr],
                                        in1=SA_s[:, cols], op=Alu.mult)
                ps2s = []
                for hb, tp in ((0, (0, 0)), (HID, (HID, 0))):
                    ps2 = ps2p.tile([F2, RW], f32, tag="ps2")
                    nc.tensor.matmul(ps2[:, :wr],
                                     lhsT=W2_s[hb : hb + HID, :],
                                     rhs=U_s[hb : hb + HID, cols],
                                     start=True, stop=True, tile_position=tp)
                    ps2s.append(ps2)
                ps2_of[ri] = ps2s
                for ps2, H2 in zip(ps2s, (H2E, H2O)):
                    nc.scalar.activation(out=H2[:, cols], in_=ps2[:, :wr],
                                         func=Act.Relu, bias=B2_s[:],
                                         scale=1.0)

            def stage_c(ri):
                (r0, wr, Dr, idxs) = meta["regions"][ri]
                cols = slice(r0, r0 + wr)
                ps3 = ps3p.tile([P, RW], f32, tag="ps3")
                for hb, H2 in ((0, H2E), (HID, H2O)):
                    nc.tensor.matmul(ps3[hb : hb + HID, :wr], lhsT=W3_s[:],
                                     rhs=H2[:, cols], start=True, stop=True)
                nc.vector.tensor_tensor(out=T3_s[:, cols], in0=ps3[:, :wr],
                                        in1=SA_s[:, cols], op=Alu.mult)
                if ri % 2 == 1 or ri >= nreg - 3 or r0 + wr == NCOL:
                    nc.sync.dma_start(T3P_d[:, stored[0] : r0 + wr],
                                      T3_s[:, stored[0] : r0 + wr])
                    stored[0] = r0 + wr

            ps_of = {}
            ps2_of = {}
            for ri, (r0, wr, Dr, idxs) in enumerate(meta["regions"]):
                split, eng_name = OFF.get(ri, (Dr, None))
                split = min(split, Dr)
                if split < Dr:
                    eng = nc.vector if eng_name == "dve" else nc.gpsimd
                    first = True
                    for si in idxs[split:]:
                        (_, d, w, soff) = meta["strips"][si]
                        strip = MSG_s[:, soff : soff + w]
                        if first:
                            eng.tensor_copy(ACC[:, r0 : r0 + w], strip)
                            first = False
                        else:
                            eng.tensor_tensor(out=ACC[:, r0 : r0 + w],
                                              in0=ACC[:, r0 : r0 + w],
                                              in1=strip, op=Alu.add)
                ps = psAp.tile([P, RW], f32, tag="psA")
                ps_of[ri] = ps
                for j, si in enumerate(idxs[:split]):
                    (_, d, w, soff) = meta["strips"][si]
                    nc.tensor.matmul(ps[:, :w], lhsT=ID_s[:],
                                     rhs=MSG_s[:, soff : soff + w],
                                     start=(j == 0),
                                     stop=(j == Dr - 1 and split == Dr))
                if split < Dr:
                    (_, d, w, soff) = meta["strips"][idxs[split]]
                    nc.tensor.matmul(ps[:, :w], lhsT=ID_s[:],
                                     rhs=ACC[:, r0 : r0 + w],
                                     start=False, stop=True)
                if ri >= 1:
                    stage_a(ri - 1)
                if ri >= 2:
                    stage_c(ri - 2)
            stage_a(nreg - 1)
            stage_c(nreg - 2)
            stage_c(nreg - 1)
    nc.compile()
    return nc


def _prog_pool(meta):
    """Per-graph max over NPG depth-major size-ranked groups; OUTF
    [HID, GPC] f32 is features x ranked graphs (host permutes back)."""
    import concourse.mybir as mybir
    import concourse.tile as tile
    f16 = mybir.dt.float16
    f32 = mybir.dt.float32
    Alu = mybir.AluOpType
    NPG, GH = meta["NPG"], meta["GH"]
    S2G, offG, POOLW = meta["S2G"], meta["offG"], meta["POOLW"]
    nc = _mk_bass()

    MSGP_d = nc.dram_tensor("MSGP", [P, POOLW], f16, kind="ExternalInput")
    OUTF_d = nc.dram_tensor("OUTF", [HID, GPC], f32, kind="ExternalOutput")

    with tile.TileContext(nc, num_cores=C) as tc:
        with (
            tc.tile_pool(name="const", bufs=1) as const,
            tc.tile_pool(name="ps", bufs=2, space="PSUM") as psp,
        ):
            mg = const.tile([P, POOLW], f16)
            OUTF_s = const.tile([HID, GPC], f32)
            for G in range(NPG):
                nc.sync.dma_start(mg[:, offG[G] : offG[G + 1]],
                                  MSGP_d[:, offG[G] : offG[G + 1]])
            for G in range(NPG):
                g0 = offG[G]
                eng = nc.vector
                v = mg[:, g0 : offG[G + 1]].rearrange(
                    "p (d g) -> p d g", g=GH)
                cur = S2G[G]
                while cur > 1:
                    h = cur // 2
                    if cur % 2:
                        eng.tensor_tensor(
                            out=v[:, 0, :], in0=v[:, 0, :],
                            in1=v[:, cur - 1, :], op=Alu.max)
                    eng.tensor_tensor(
                        out=v[:, 0:h, :], in0=v[:, 0:h, :],
                        in1=v[:, h : 2 * h, :], op=Alu.max)
                    cur = h
                bot = psp.tile([HID, GH], f32, tag="bot")
                nc.vector.tensor_copy(bot[:], mg[HID:P, g0 : g0 + GH])
                nc.vector.tensor_tensor(
                    out=OUTF_s[:, G * GH : (G + 1) * GH],
                    in0=mg[0:HID, g0 : g0 + GH],
                    in1=bot[:], op=Alu.max)
            nc.sync.dma_start(OUTF_d[:], OUTF_s[:])
    nc.compile()
    return nc


# --------------------------------------------------------------------------
# Entry point
# --------------------------------------------------------------------------

_RUN_KWARGS = {}
_EXEC_NS = []
_PROFILE = False


def _stage_msgs(T_full, srcmap_c):
    """[N+1, HID] table + [2, SLOTS] slot->row map -> [128, SLOTS]."""
    top = T_full[srcmap_c[0]].T      # [64, SLOTS]
    bot = T_full[srcmap_c[1]].T
    return np.ascontiguousarray(np.concatenate([top, bot], axis=0))


def _assemble(prep, parts_E, parts_O, dtype):
    """Per-core [64, NCOL] half strips -> full [N+1, HID] table."""
    T_full = np.zeros((N + 1, HID), dtype)
    for c in range(C):
        tops, bots = prep["tops"][c], prep["bots"][c]
        mE, mO = tops >= 0, bots >= 0
        T_full[tops[mE]] = parts_E[c][:, mE].T
        T_full[bots[mO]] = parts_O[c][:, mO].T
    return T_full


def kernel(data, edge_index, batch, W1, b1, W2, b2, W3, b3):
    from concourse.bass_utils import run_bass_kernel_spmd

    data = np.asarray(data, dtype=np.float32)
    edge_index = np.asarray(edge_index, dtype=np.int32)
    batch_np = np.asarray(batch, dtype=np.int32)

    prep = _host_prep(edge_index, batch_np)
    meta = prep["meta"]
    NCOL = meta["NCOL"]

    IDENT8 = np.eye(P, dtype=F8)
    W1f = np.asarray(W1, np.float32).astype(F16)            # [128, 64]
    W2f = np.asarray(W2, np.float32).astype(F16)            # [64, 128]
    W3f = (np.asarray(W3, np.float32) * (M2 * M3)).astype(F16)
    B1K = (np.tile(np.asarray(b1, np.float32), 2) * (M1 * M2))[:, None].copy()
    B2r = np.asarray(b2, np.float32)[:, None].copy()
    B3r = np.tile(np.asarray(b3, np.float32), 2)[:, None].copy()

    Xx = np.concatenate([data, np.zeros((1, IN_DIM), np.float32)], axis=0)
    XT = np.empty((C, IN_DIM, 2 * NCOL), F16)
    for c in range(C):
        XT[c, :, :NCOL] = Xx[prep["topsx"][c]].T
        XT[c, :, NCOL:] = Xx[prep["botsx"][c]].T

    cores = list(range(C))
    del _EXEC_NS[:]

    def run(nc, in_maps):
        if _PROFILE:
            from concourse.timeline_sim import TimelineSim
            _EXEC_NS.append(TimelineSim(nc, require_finite=False).simulate())
        res = run_bass_kernel_spmd(nc, in_maps, cores, **_RUN_KWARGS)
        if res.exec_time_ns is not None:
            _EXEC_NS.append(res.exec_time_ns)
        return res.results

    def e2(const):
        m = np.zeros((2, P), F16)
        m[0, :HID] = const
        m[1, HID:] = const
        return m

    # ---- L1: T1 = (s*M1) * (X @ W1) ----
    r1 = run(_prog_l1(meta),
             [{"XT": np.ascontiguousarray(XT[c]), "W1": W1f,
               "SROW": np.ascontiguousarray(prep["SROW"][c]),
               "E2": e2(M1)}
              for c in range(C)])
    T1 = _assemble(prep,
                   [np.asarray(r1[c]["T1P"])[0:HID] for c in range(C)],
                   [np.asarray(r1[c]["T1P"])[HID:P] for c in range(C)], F8)

    # ---- L2: T2 = (s*M2)*relu(s*Agg(T1)/M1 + b1) ----
    r2 = run(_prog_agg(meta, 2),
             [{"MSG": _stage_msgs(T1, prep["srcmap"][c]),
               "SROW": np.ascontiguousarray(prep["SROW"][c]),
               "E2": e2(1.0 / M1), "BK": B1K, "ID": IDENT8}
              for c in range(C)])
    T2 = _assemble(prep,
                   [np.asarray(r2[c]["OUT"])[0:HID] for c in range(C)],
                   [np.asarray(r2[c]["OUT"])[HID:P] for c in range(C)], F8)

    # ---- L3: T3 = (s*M3)*(relu((s*Agg(T2)/M2)@W2 + b2)@W3) ----
    r3 = run(_prog_l3(meta),
             [{"MSG": _stage_msgs(T2, prep["srcmap"][c]),
               "SA": np.ascontiguousarray(prep["SA3"][c]),
               "W2": np.concatenate([W2f, W2f], axis=0), "W3": W3f,
               "B2": B2r, "ID": IDENT8}
              for c in range(C)])
    T3 = _assemble(prep,
                   [np.asarray(r3[c]["T3P"])[0:HID] for c in range(C)],
                   [np.asarray(r3[c]["T3P"])[HID:P] for c in range(C)], F8)

    # ---- L4: H3 = s*Agg(T3)/M3 + b3 ----
    r4 = run(_prog_agg(meta, 4),
             [{"MSG": _stage_msgs(T3, prep["srcmap"][c]),
               "SROW": np.ascontiguousarray(prep["SROW"][c]),
               "E2": e2(1.0 / M3), "BK": B3r, "ID": IDENT8}
              for c in range(C)])
    H3 = _assemble(prep,
                   [np.asarray(r4[c]["OUT"])[0:HID] for c in range(C)],
                   [np.asarray(r4[c]["OUT"])[HID:P] for c in range(C)], F16)
    H3[N] = np.float16(-60000.0)     # pad row for the pool staging

    # ---- L5: per-graph max pool ----
    r5 = run(_prog_pool(meta),
             [{"MSGP": _stage_msgs(H3, prep["poolmap"][c])}
              for c in range(C)])
    out = np.empty((N_GRAPHS, HID), np.float32)
    for c in range(C):
        of = np.asarray(r5[c]["OUTF"]).astype(np.float32).T   # ranked graphs
        out[c * GPC + prep["grank"][c]] = of
    out[prep["cnt"].reshape(-1) == 0] = -np.inf
    return out


# revision 43
# speedup vs baseline: 1.0029x; 1.0029x over previous
"""Trainium2 Bass kernel for a 3-layer GCN encoder with global max pool.

Strategy (8 NeuronCores, SPMD, 5 launches, host staging between launches):
  - Nodes partitioned graph-wise (graph g -> core g//64). The host only MOVES
    device-computed bytes between launches (gather rows into padded message
    tables); every FLOP runs on device.
  - Aggregation layers stage per-edge messages in fp8 (e3m4) with fixed
    power-free scale factors folded into the device-side s-tables, halving
    HBM traffic vs fp16.
  - The aggregation sum runs on the Tensor engine: identity-weight matmuls
    accumulate message strips into PSUM (start/stop prefix accumulation).
    Columns (node pairs) are sorted by descending in-degree so the set of
    columns with a message at depth d is a prefix; strips are stored
    region-major (one 512-column PSUM bank region at a time) so PSUM holds
    each region until its sum completes.
  - Post-ops per region: DVE multiplies PSUM by the s-table; Activation
    applies (scaled) bias+relu; DVE writes the next layer's pre-scaled
    fp8 table directly.
  - Launches: L1  T1 = (s*M1) * (X @ W1)                 [fp8 out]
              L2  T2 = (s*M2) * relu(s*Agg(T1)/M1 + b1)  [fp8 out]
              L3  T3 = (s*M3) * (relu((s*Agg(T2)/M2)@W2 + b2)@W3)
              L4  H3 = s*Agg(T3)/M3 + b3                 [fp16 out]
              L5  per-graph max pool over H3 (depth-major staged layout)
"""

import numpy as np
import ml_dtypes

N = 50000
IN_DIM = 128
HID = 64
F2 = 2 * HID
N_GRAPHS = 512
C = 8
P = 128
GPC = N_GRAPHS // C
RW = 512            # psum region width (columns)
F16 = np.float16
F8 = ml_dtypes.float8_e3m4

M1, M2, M3 = 5.0, 11.0, 44.0   # staging scale factors (fold into s-tables)

# aggregation offload: region -> (PE strip count, engine for the rest)
_L3_OFF = {1: (10, "dve"), 2: (12, "pool"), 3: (10, "dve")}
_AGG_OFF = {}


# --------------------------------------------------------------------------
# Host-side preprocessing (graph structure only - no feature arithmetic)
# --------------------------------------------------------------------------

def _host_prep(edge_index, batch):
    src = np.asarray(edge_index[0], dtype=np.int64)
    dst = np.asarray(edge_index[1], dtype=np.int64)
    batch = np.asarray(batch, dtype=np.int64)
    core_of = batch // GPC

    indeg = np.bincount(dst, minlength=N)
    k = indeg + 1                     # slots per node incl. self loop
    s = (1.0 / np.sqrt(k.astype(np.float64))).astype(np.float32)

    # in-neighbor lists grouped by dst
    eorder = np.argsort(dst, kind="stable")
    esrc = src[eorder]
    estart = np.zeros(N + 1, np.int64)
    np.cumsum(np.bincount(dst, minlength=N), out=estart[1:])

    # per-core node order: descending k, paired (2i, 2i+1) into columns
    orders = []
    for c in range(C):
        nodes = np.nonzero(core_of == c)[0]
        orders.append(nodes[np.argsort(-k[nodes], kind="stable")])
    NCOL = max((len(o) + 1) // 2 for o in orders)

    tops = np.full((C, NCOL), -1, np.int64)
    bots = np.full((C, NCOL), -1, np.int64)
    for c in range(C):
        o = orders[c]
        tops[c, : len(o[0::2])] = o[0::2]
        bots[c, : len(o[1::2])] = o[1::2]
    topsx = np.where(tops >= 0, tops, N)
    botsx = np.where(bots >= 0, bots, N)

    # column depth = max over cores of max(k_top, k_bot); >=1 (self loop)
    kk = np.concatenate([k, [0]])
    D_col = np.maximum(kk[topsx], kk[botsx]).max(axis=0)
    D_col = np.maximum(D_col, 1)
    assert (np.diff(D_col) <= 0).all()
    DMAX = int(D_col[0])
    n_d = np.array([(D_col > d).sum() for d in range(DMAX)], np.int64)

    # region widths: full PSUM banks, with the remainder split into a
    # shrinking taper so the final post-op chains are short
    widths = []
    rem = NCOL
    while rem > 704:
        widths.append(RW)
        rem -= RW
    if rem > 384:
        w1 = (rem * 33 // 64) & ~31
        w2 = ((rem - w1) * 3 // 5) & ~31
        widths += [w1, w2, rem - w1 - w2]
    elif rem > 160:
        w1 = (rem * 3 // 5) & ~31
        widths += [w1, rem - w1]
    else:
        widths.append(rem)

    # region-major strips: (region_col0, d, w, slot_off)
    strips = []
    off = 0
    regions = []          # (col0, width, D_r, [strip indices])
    r0 = 0
    for wr in widths:
        Dr = int(D_col[r0])
        idxs = []
        for d in range(Dr):
            w = int(min(n_d[d] - r0, wr))
            assert w > 0
            idxs.append(len(strips))
            strips.append((r0, d, w, off))
            off += w
        regions.append((r0, wr, Dr, idxs))
        r0 += wr
    SLOTS = off

    # slot -> source node maps (N = zero row) for tops/bottoms
    indegx = np.concatenate([indeg, [0]])
    estartx = np.concatenate([estart[:-1], [0]])
    srcmap = np.full((C, 2, SLOTS), N, np.int64)
    for (r0, d, w, soff) in strips:
        for c in range(C):
            for half, nodes_h in ((0, topsx[c]), (1, botsx[c])):
                v = nodes_h[r0 : r0 + w]
                if d == 0:
                    srcmap[c, half, soff : soff + w] = v
                else:
                    sel = (d <= indegx[v]) & (v < N)
                    tgt = srcmap[c, half, soff : soff + w]
                    tgt[sel] = esrc[estartx[v[sel]] + d - 1]

    # s rows [C, 2, NCOL] fp16 (top/bot); the launch-specific scale constant
    # rides in the tiny broadcast matrix E2 instead of a full table.
    sx = np.concatenate([s, [0.0]]).astype(F16)
    SROW = np.stack([sx[topsx], sx[botsx]], axis=1)    # [C, 2, NCOL]

    # L3 keeps a full [128, NCOL] table (PE there is the bottleneck)
    sx3 = np.concatenate([s * (1.0 / M2), [0.0]]).astype(F16)
    top = sx3[topsx][:, None, :].repeat(HID, axis=1)
    bot = sx3[botsx][:, None, :].repeat(HID, axis=1)
    SA3 = np.concatenate([top, bot], axis=1)           # [C, 128, NCOL]

    # pooling: graphs ranked by size per core, split into NPG groups of GH;
    # group G is depth-major: column offG + d*GH + j = d-th node pair of the
    # (G*GH+j)-th largest graph.  Per-group depth S2G trims the rectangle.
    NPG = 4
    GH = GPC // NPG
    gl = batch % GPC
    cnt = np.zeros((C, GPC), np.int64)
    np.add.at(cnt, (core_of, gl), 1)
    grank = np.argsort(-cnt, axis=1, kind="stable")     # [C, GPC] rank->graph
    pairs = -(-cnt // 2)
    S2G = []
    for G in range(NPG):
        S2G.append(int(max(pairs[c, grank[c, G * GH]] for c in range(C))))
    offG = np.zeros(NPG + 1, np.int64)
    np.cumsum(np.array(S2G) * GH, out=offG[1:])
    POOLW = int(offG[-1])
    poolmap = np.full((C, 2, POOLW), N, np.int64)
    for c in range(C):
        for j in range(GPC):
            g = grank[c, j]
            nodes = np.nonzero((core_of == c) & (gl == g))[0]
            e = nodes[0::2]
            o = nodes[1::2]
            G = j // GH
            base = int(offG[G]) + (j % GH)
            poolmap[c, 0, base : base + S2G[G] * GH : GH][: len(e)] = e
            poolmap[c, 1, base : base + S2G[G] * GH : GH][: len(o)] = o

    meta = dict(NCOL=NCOL, SLOTS=SLOTS, strips=strips, regions=regions,
                NPG=NPG, GH=GH, S2G=S2G, offG=[int(x) for x in offG],
                POOLW=POOLW)
    return dict(meta=meta, tops=tops, bots=bots, topsx=topsx, botsx=botsx,
                srcmap=srcmap, poolmap=poolmap, cnt=cnt, grank=grank,
                SROW=SROW, SA3=SA3)


# --------------------------------------------------------------------------
# Bass programs
# --------------------------------------------------------------------------

def _mk_bass():
    import concourse.bacc as bacc
    return bacc.Bacc(None)


def _msg_chunks(strips, SLOTS, first=1400, later=3600):
    """Split the slot axis into DMA chunks at strip boundaries."""
    cuts = []
    target = first
    for (r0, d, w, soff) in strips:
        end = soff + w
        if end >= target:
            cuts.append(end)
            target = end + later
    if not cuts or cuts[-1] != SLOTS:
        cuts.append(SLOTS)
    out = []
    a = 0
    for b in cuts:
        out.append((a, b))
        a = b
    return out


def _prog_agg(meta, layer):
    """L2 (layer==2): OUT = (s*M2)*relu((M1*M2)*(A*s/M1) + (M1*M2)*b1), fp8
       L4 (layer==4): OUT = A*s/M3 + b3, fp16"""
    import concourse.mybir as mybir
    import concourse.tile as tile
    f8 = mybir.dt.float8e3
    f16 = mybir.dt.float16
    f32 = mybir.dt.float32
    Alu = mybir.AluOpType
    Act = mybir.ActivationFunctionType
    NCOL, SLOTS = meta["NCOL"], meta["SLOTS"]
    nc = _mk_bass()

    MSG_d = nc.dram_tensor("MSG", [P, SLOTS], f8, kind="ExternalInput")
    SROW_d = nc.dram_tensor("SROW", [2, NCOL], f16, kind="ExternalInput")
    E2_d = nc.dram_tensor("E2", [2, P], f16, kind="ExternalInput")
    BK_d = nc.dram_tensor("BK", [P, 1], f32, kind="ExternalInput")
    ID_d = nc.dram_tensor("ID", [P, P], f8, kind="ExternalInput")
    OUT_d = nc.dram_tensor("OUT", [P, NCOL], f8 if layer == 2 else f16,
                           kind="ExternalOutput")

    OFF = dict(_AGG_OFF)

    with tile.TileContext(nc, num_cores=C) as tc:
        with (
            tc.tile_pool(name="const", bufs=1) as const,
            tc.tile_pool(name="ps", bufs=3, space="PSUM") as psp,
        ):
            ID_s = const.tile([P, P], f8)
            nc.sync.dma_start(ID_s[:], ID_d[:])
            SROW_s = const.tile([2, NCOL], f16)
            nc.sync.dma_start(SROW_s[:], SROW_d[:])
            E2_s = const.tile([2, P], f16)
            nc.sync.dma_start(E2_s[:], E2_d[:])
            BK_s = const.tile([P, 1], f32)
            nc.sync.dma_start(BK_s[:], BK_d[:])
            MSG_s = const.tile([P, SLOTS], f8)
            chunks = _msg_chunks(meta["strips"], SLOTS)
            for (a, b) in chunks:
                nc.sync.dma_start(MSG_s[:, a:b], MSG_d[:, a:b])
            SA_s = const.tile([P, NCOL], f16)
            U_s = const.tile([P, NCOL], f16)
            H_s = const.tile([P, NCOL], f16)
            OUT_s = const.tile([P, NCOL], f8 if layer == 2 else f16)
            ACC = const.tile([P, NCOL], f16)

            # PE warm-up during the first MSG DMA (pstate ramp); the s-row
            # broadcast matmuls are interleaved into region 0's strip
            # stream below, filling PE's early chunk-wait gaps
            wm = psp.tile([P, RW], f32, tag="ps")
            for i in range(6):
                nc.tensor.matmul(wm[:, :P], lhsT=ID_s[:], rhs=ID_s[:],
                                 start=(i == 0), stop=(i == 5))
            bcast_jobs = list(range(0, NCOL, RW))

            def emit_bcast():
                a = bcast_jobs.pop(0)
                w = min(RW, NCOL - a)
                psSA = psp.tile([P, RW], f32, tag="psb")
                nc.tensor.matmul(psSA[:, :w], lhsT=E2_s[:],
                                 rhs=SROW_s[:, a : a + w],
                                 start=True, stop=True)
                nc.scalar.activation(out=SA_s[:, a : a + w],
                                     in_=psSA[:, :w], func=Act.Copy,
                                     bias=0.0, scale=1.0)

            nreg = len(meta["regions"])
            stored = [0]

            def fin(ri):
                # final OUT mult + store, emitted one region late so the
                # engines pipeline across the last two regions
                (r0, wr, Dr, idxs) = meta["regions"][ri]
                cols = slice(r0, r0 + wr)
                if layer == 2:
                    eng = nc.gpsimd if ri in (1, 3) else nc.vector
                    eng.tensor_tensor(out=OUT_s[:, cols], in0=H_s[:, cols],
                                      in1=SA_s[:, cols], op=Alu.mult)
                if ri % 2 == 1 or ri >= nreg - 3 or r0 + wr == NCOL:
                    nc.sync.dma_start(OUT_d[:, stored[0] : r0 + wr],
                                      OUT_s[:, stored[0] : r0 + wr])
                    stored[0] = r0 + wr

            for ri, (r0, wr, Dr, idxs) in enumerate(meta["regions"]):
                split, eng_name = OFF.get(ri, (Dr, None))
                split = min(split, Dr)
                if split < Dr:
                    aeng = nc.vector if eng_name == "dve" else nc.gpsimd
                    first = True
                    for si in idxs[split:]:
                        (_, d, w, soff) = meta["strips"][si]
                        strip = MSG_s[:, soff : soff + w]
                        if first:
                            aeng.tensor_copy(ACC[:, r0 : r0 + w], strip)
                            first = False
                        else:
                            aeng.tensor_tensor(out=ACC[:, r0 : r0 + w],
                                               in0=ACC[:, r0 : r0 + w],
                                               in1=strip, op=Alu.add)
                ps = psp.tile([P, RW], f32, tag="ps")
                for j, si in enumerate(idxs[:split]):
                    (_, d, w, soff) = meta["strips"][si]
                    nc.tensor.matmul(ps[:, :w], lhsT=ID_s[:],
                                     rhs=MSG_s[:, soff : soff + w],
                                     start=(j == 0),
                                     stop=(j == Dr - 1 and split == Dr))
                    if ri <= 1 and j >= 2 and bcast_jobs:
                        emit_bcast()
                while ri == 1 and bcast_jobs:
                    emit_bcast()
                if split < Dr:
                    (_, d, w, soff) = meta["strips"][idxs[split]]
                    nc.tensor.matmul(ps[:, :w], lhsT=ID_s[:],
                                     rhs=ACC[:, r0 : r0 + w],
                                     start=False, stop=True)
                cols = slice(r0, r0 + wr)
                nc.vector.tensor_tensor(out=U_s[:, cols], in0=ps[:, :wr],
                                        in1=SA_s[:, cols], op=Alu.mult)
                if layer == 2:
                    nc.scalar.activation(out=H_s[:, cols], in_=U_s[:, cols],
                                         func=Act.Relu, bias=BK_s[:],
                                         scale=float(M1 * M2))
                else:
                    nc.scalar.activation(out=OUT_s[:, cols], in_=U_s[:, cols],
                                         func=Act.Identity, bias=BK_s[:],
                                         scale=1.0)
                if ri >= 1:
                    fin(ri - 1)
            fin(nreg - 1)
    nc.compile()
    return nc


def _prog_l1(meta):
    """T1 = (s*M1) * (X @ W1), two [64, NCOL] fp8 half strips."""
    import concourse.mybir as mybir
    import concourse.tile as tile
    f8 = mybir.dt.float8e3
    f16 = mybir.dt.float16
    f32 = mybir.dt.float32
    Alu = mybir.AluOpType
    Act = mybir.ActivationFunctionType
    NCOL = meta["NCOL"]
    nc = _mk_bass()

    XT_d = nc.dram_tensor("XT", [IN_DIM, 2 * NCOL], f16, kind="ExternalInput")
    W1_d = nc.dram_tensor("W1", [IN_DIM, HID], f16, kind="ExternalInput")
    SROW_d = nc.dram_tensor("SROW", [2, NCOL], f16, kind="ExternalInput")
    E2_d = nc.dram_tensor("E2", [2, P], f16, kind="ExternalInput")
    T1P_d = nc.dram_tensor("T1P", [P, NCOL], f8, kind="ExternalOutput")

    with tile.TileContext(nc, num_cores=C) as tc:
        with (
            tc.tile_pool(name="const", bufs=1) as const,
            tc.tile_pool(name="ps", bufs=4, space="PSUM") as psp,
        ):
            W1_s = const.tile([IN_DIM, HID], f16)
            nc.sync.dma_start(W1_s[:], W1_d[:])
            SROW_s = const.tile([2, NCOL], f16)
            nc.sync.dma_start(SROW_s[:], SROW_d[:])
            E2_s = const.tile([2, P], f16)
            nc.sync.dma_start(E2_s[:], E2_d[:])
            XT_s = const.tile([IN_DIM, 2 * NCOL], f16)
            SRT1_s = const.tile([P, NCOL], f16)
            cuts = [RW] + list(range(2 * RW, NCOL, 2 * RW)) + [NCOL]
            chunks = []
            a = 0
            for b in cuts:
                if b > a:
                    chunks.append((a, b))
                    a = b
            for ci, (a, b) in enumerate(chunks):
                nc.sync.dma_start(XT_s[:, a:b], XT_d[:, a:b])
                nc.sync.dma_start(XT_s[:, NCOL + a : NCOL + b],
                                  XT_d[:, NCOL + a : NCOL + b])
            T1_s = const.tile([P, NCOL], f8)
            V_s = const.tile([P, NCOL], f16)

            # warm up PE, then broadcast the s-row into SRT1 via PE + Act
            wm = psp.tile([P, RW], f32, tag="ps")
            for i in range(4):
                nc.tensor.matmul(wm[0:HID, :HID], lhsT=W1_s[:], rhs=W1_s[:],
                                 start=(i == 0), stop=(i == 3))
            for a in range(0, NCOL, RW):
                w = min(RW, NCOL - a)
                psSA = psp.tile([P, RW], f32, tag="ps")
                nc.tensor.matmul(psSA[:, :w], lhsT=E2_s[:],
                                 rhs=SROW_s[:, a : a + w],
                                 start=True, stop=True)
                nc.scalar.activation(out=SRT1_s[:, a : a + w],
                                     in_=psSA[:, :w], func=Act.Copy,
                                     bias=0.0, scale=1.0)

            stored = 0
            nflows = -(-NCOL // RW)
            for i in range(nflows):
                a = i * RW
                w = min(RW, NCOL - a)
                ps = psp.tile([P, RW], f32, tag="ps")
                for half in range(2):
                    nc.tensor.matmul(
                        ps[half * HID : half * HID + HID, :w], lhsT=W1_s[:],
                        rhs=XT_s[:, half * NCOL + a : half * NCOL + a + w],
                        start=True, stop=True)
                if i in (2, 4):
                    # relieve DVE: Act copies PSUM out, Pool applies the scale
                    nc.scalar.activation(out=V_s[:, a : a + w],
                                         in_=ps[:, :w], func=Act.Copy,
                                         bias=0.0, scale=1.0)
                    nc.gpsimd.tensor_tensor(
                        out=T1_s[:, a : a + w], in0=V_s[:, a : a + w],
                        in1=SRT1_s[:, a : a + w], op=Alu.mult)
                else:
                    nc.vector.tensor_tensor(
                        out=T1_s[:, a : a + w], in0=ps[:, :w],
                        in1=SRT1_s[:, a : a + w], op=Alu.mult)
                if i % 2 == 1 or a + w == NCOL:
                    nc.scalar.dma_start(T1P_d[:, stored : a + w],
                                        T1_s[:, stored : a + w])
                    stored = a + w
    nc.compile()
    return nc


def _prog_l3(meta):
    """T3 = (s*M3)*(relu((s*Agg(T2)/M2)@W2 + b2)@W3), fp8 half strips.
    W3 input is pre-scaled by M2*M3 on the host."""
    import concourse.mybir as mybir
    import concourse.tile as tile
    f8 = mybir.dt.float8e3
    f16 = mybir.dt.float16
    f32 = mybir.dt.float32
    Alu = mybir.AluOpType
    Act = mybir.ActivationFunctionType
    NCOL, SLOTS = meta["NCOL"], meta["SLOTS"]
    nc = _mk_bass()

    MSG_d = nc.dram_tensor("MSG", [P, SLOTS], f8, kind="ExternalInput")
    SA_d = nc.dram_tensor("SA", [P, NCOL], f16, kind="ExternalInput")
    W2_d = nc.dram_tensor("W2", [P, F2], f16, kind="ExternalInput")
    W3_d = nc.dram_tensor("W3", [F2, HID], f16, kind="ExternalInput")
    B2_d = nc.dram_tensor("B2", [F2, 1], f32, kind="ExternalInput")
    ID_d = nc.dram_tensor("ID", [P, P], f8, kind="ExternalInput")
    T3P_d = nc.dram_tensor("T3P", [P, NCOL], f8, kind="ExternalOutput")

    # deep strips of these regions are tree-summed on DVE/Pool into an fp16
    # partial; PE folds the partial into PSUM with one extra pass
    OFF = dict(_L3_OFF)

    with tile.TileContext(nc, num_cores=C) as tc:
        with (
            tc.tile_pool(name="const", bufs=1) as const,
            tc.tile_pool(name="psA", bufs=2, space="PSUM") as psAp,
            tc.tile_pool(name="ps2", bufs=4, space="PSUM") as ps2p,
            tc.tile_pool(name="ps3", bufs=2, space="PSUM") as ps3p,
        ):
            ID_s = const.tile([P, P], f8)
            nc.sync.dma_start(ID_s[:], ID_d[:])
            MSG_s = const.tile([P, SLOTS], f8)
            chunks = _msg_chunks(meta["strips"], SLOTS)
            SA_s = const.tile([P, NCOL], f16)
            W2_s = const.tile([P, F2], f16)
            W3_s = const.tile([F2, HID], f16)
            B2_s = const.tile([F2, 1], f32)
            for ci, (a, b) in enumerate(chunks):
                nc.sync.dma_start(MSG_s[:, a:b], MSG_d[:, a:b])
                if ci == 0:
                    nc.sync.dma_start(W2_s[:], W2_d[:])
                    nc.sync.dma_start(W3_s[:], W3_d[:])
                    nc.sync.dma_start(B2_s[:], B2_d[:])
                elif ci == 2:
                    nc.sync.dma_start(SA_s[:], SA_d[:])
            U_s = const.tile([P, NCOL], f16)
            H2E = const.tile([F2, NCOL], f16)
            H2O = const.tile([F2, NCOL], f16)
            T3_s = const.tile([P, NCOL], f8)
            ACC = const.tile([P, NCOL], f16)

            # PE warm-up during the first MSG DMA
            wm = psAp.tile([P, RW], f32, tag="psA")
            for i in range(8):
                nc.tensor.matmul(wm[:, :P], lhsT=ID_s[:], rhs=ID_s[:],
                                 start=(i == 0), stop=(i == 7))

            stored = [0]
            nreg = len(meta["regions"])

            def stage_a(ri):
                (r0, wr, Dr, idxs) = meta["regions"][ri]
                ps = ps_of[ri]
                cols = slice(r0, r0 + wr)
                nc.vector.tensor_tensor(out=U_s[:, cols], in0=ps[:, :wr],
                                        in1=SA_s[:, cols], op=Alu.mult)
                ps2s = []
                for hb, tp in ((0, (0, 0)), (HID, (HID, 0))):
                    ps2 = ps2p.tile([F2, RW], f32, tag="ps2")
                    nc.tensor.matmul(ps2[:, :wr],
                                     lhsT=W2_s[hb : hb + HID, :],
                                     rhs=U_s[hb : hb + HID, cols],
                                     start=True, stop=True, tile_position=tp)
                    ps2s.append(ps2)
                ps2_of[ri] = ps2s
                for ps2, H2 in zip(ps2s, (H2E, H2O)):
                    nc.scalar.activation(out=H2[:, cols], in_=ps2[:, :wr],
                                         func=Act.Relu, bias=B2_s[:],
                                         scale=1.0)

            def stage_c(ri):
                (r0, wr, Dr, idxs) = meta["regions"][ri]
                cols = slice(r0, r0 + wr)
                ps3 = ps3p.tile([P, RW], f32, tag="ps3")
                for hb, H2 in ((0, H2E), (HID, H2O)):
                    nc.tensor.matmul(ps3[hb : hb + HID, :wr], lhsT=W3_s[:],
                                     rhs=H2[:, cols], start=True, stop=True)
                nc.vector.tensor_tensor(out=T3_s[:, cols], in0=ps3[:, :wr],
                                        in1=SA_s[:, cols], op=Alu.mult)
                if ri % 2 == 1 or ri >= nreg - 3 or r0 + wr == NCOL:
                    nc.sync.dma_start(T3P_d[:, stored[0] : r0 + wr],
                                      T3_s[:, stored[0] : r0 + wr])
                    stored[0] = r0 + wr

            ps_of = {}
            ps2_of = {}
            for ri, (r0, wr, Dr, idxs) in enumerate(meta["regions"]):
                split, eng_name = OFF.get(ri, (Dr, None))
                split = min(split, Dr)
                if split < Dr:
                    eng = nc.vector if eng_name == "dve" else nc.gpsimd
                    first = True
                    for si in idxs[split:]:
                        (_, d, w, soff) = meta["strips"][si]
                        strip = MSG_s[:, soff : soff + w]
                        if first:
                            eng.tensor_copy(ACC[:, r0 : r0 + w], strip)
                            first = False
                        else:
                            eng.tensor_tensor(out=ACC[:, r0 : r0 + w],
                                              in0=ACC[:, r0 : r0 + w],
                                              in1=strip, op=Alu.add)
                ps = psAp.tile([P, RW], f32, tag="psA")
                ps_of[ri] = ps
                for j, si in enumerate(idxs[:split]):
                    (_, d, w, soff) = meta["strips"][si]
                    nc.tensor.matmul(ps[:, :w], lhsT=ID_s[:],
                                     rhs=MSG_s[:, soff : soff + w],
                                     start=(j == 0),
                                     stop=(j == Dr - 1 and split == Dr))
                if split < Dr:
                    (_, d, w, soff) = meta["strips"][idxs[split]]
                    nc.tensor.matmul(ps[:, :w], lhsT=ID_s[:],
                                     rhs=ACC[:, r0 : r0 + w],
                                     start=False, stop=True)
                if ri >= 1:
                    stage_a(ri - 1)
                if ri >= 2:
                    stage_c(ri - 2)
            stage_a(nreg - 1)
            stage_c(nreg - 2)
            stage_c(nreg - 1)
    nc.compile()
    return nc


def _prog_pool(meta):
    """Per-graph max over NPG depth-major size-ranked groups; OUTF
    [HID, GPC] f32 is features x ranked graphs (host permutes back)."""
    import concourse.mybir as mybir
    import concourse.tile as tile
    f16 = mybir.dt.float16
    f32 = mybir.dt.float32
    Alu = mybir.AluOpType
    NPG, GH = meta["NPG"], meta["GH"]
    S2G, offG, POOLW = meta["S2G"], meta["offG"], meta["POOLW"]
    nc = _mk_bass()

    MSGP_d = nc.dram_tensor("MSGP", [P, POOLW], f16, kind="ExternalInput")
    OUTF_d = nc.dram_tensor("OUTF", [HID, GPC], f32, kind="ExternalOutput")

    with tile.TileContext(nc, num_cores=C) as tc:
        with (
            tc.tile_pool(name="const", bufs=1) as const,
            tc.tile_pool(name="ps", bufs=2, space="PSUM") as psp,
        ):
            mg = const.tile([P, POOLW], f16)
            OUTF_s = const.tile([HID, GPC], f32)
            for G in range(NPG):
                nc.sync.dma_start(mg[:, offG[G] : offG[G + 1]],
                                  MSGP_d[:, offG[G] : offG[G + 1]])
            for G in range(NPG):
                g0 = offG[G]
                eng = nc.vector
                v = mg[:, g0 : offG[G + 1]].rearrange(
                    "p (d g) -> p d g", g=GH)
                cur = S2G[G]
                while cur > 1:
                    h = cur // 2
                    if cur % 2:
                        eng.tensor_tensor(
                            out=v[:, 0, :], in0=v[:, 0, :],
                            in1=v[:, cur - 1, :], op=Alu.max)
                    eng.tensor_tensor(
                        out=v[:, 0:h, :], in0=v[:, 0:h, :],
                        in1=v[:, h : 2 * h, :], op=Alu.max)
                    cur = h
                bot = psp.tile([HID, GH], f32, tag="bot")
                nc.vector.tensor_copy(bot[:], mg[HID:P, g0 : g0 + GH])
                nc.vector.tensor_tensor(
                    out=OUTF_s[:, G * GH : (G + 1) * GH],
                    in0=mg[0:HID, g0 : g0 + GH],
                    in1=bot[:], op=Alu.max)
            nc.sync.dma_start(OUTF_d[:], OUTF_s[:])
    nc.compile()
    return nc


# --------------------------------------------------------------------------
# Entry point
# --------------------------------------------------------------------------

_RUN_KWARGS = {}
_EXEC_NS = []
_PROFILE = False


def _stage_msgs(T_full, srcmap_c):
    """[N+1, HID] table + [2, SLOTS] slot->row map -> [128, SLOTS]."""
    top = T_full[srcmap_c[0]].T      # [64, SLOTS]
    bot = T_full[srcmap_c[1]].T
    return np.ascontiguousarray(np.concatenate([top, bot], axis=0))


def _assemble(prep, parts_E, parts_O, dtype):
    """Per-core [64, NCOL] half strips -> full [N+1, HID] table."""
    T_full = np.zeros((N + 1, HID), dtype)
    for c in range(C):
        tops, bots = prep["tops"][c], prep["bots"][c]
        mE, mO = tops >= 0, bots >= 0
        T_full[tops[mE]] = parts_E[c][:, mE].T
        T_full[bots[mO]] = parts_O[c][:, mO].T
    return T_full


def kernel(data, edge_index, batch, W1, b1, W2, b2, W3, b3):
    from concourse.bass_utils import run_bass_kernel_spmd

    data = np.asarray(data, dtype=np.float32)
    edge_index = np.asarray(edge_index, dtype=np.int32)
    batch_np = np.asarray(batch, dtype=np.int32)

    prep = _host_prep(edge_index, batch_np)
    meta = prep["meta"]
    NCOL = meta["NCOL"]

    IDENT8 = np.eye(P, dtype=F8)
    W1f = np.asarray(W1, np.float32).astype(F16)            # [128, 64]
    W2f = np.asarray(W2, np.float32).astype(F16)            # [64, 128]
    W3f = (np.asarray(W3, np.float32) * (M2 * M3)).astype(F16)
    B1K = (np.tile(np.asarray(b1, np.float32), 2) * (M1 * M2))[:, None].copy()
    B2r = np.asarray(b2, np.float32)[:, None].copy()
    B3r = np.tile(np.asarray(b3, np.float32), 2)[:, None].copy()

    Xx = np.concatenate([data, np.zeros((1, IN_DIM), np.float32)], axis=0)
    XT = np.empty((C, IN_DIM, 2 * NCOL), F16)
    for c in range(C):
        XT[c, :, :NCOL] = Xx[prep["topsx"][c]].T
        XT[c, :, NCOL:] = Xx[prep["botsx"][c]].T

    cores = list(range(C))
    del _EXEC_NS[:]

    def run(nc, in_maps):
        if _PROFILE:
            from concourse.timeline_sim import TimelineSim
            _EXEC_NS.append(TimelineSim(nc, require_finite=False).simulate())
        res = run_bass_kernel_spmd(nc, in_maps, cores, **_RUN_KWARGS)
        if res.exec_time_ns is not None:
            _EXEC_NS.append(res.exec_time_ns)
        return res.results

    def e2(const):
        m = np.zeros((2, P), F16)
        m[0, :HID] = const
        m[1, HID:] = const
        return m

    # ---- L1: T1 = (s*M1) * (X @ W1) ----
    r1 = run(_prog_l1(meta),
             [{"XT": np.ascontiguousarray(XT[c]), "W1": W1f,
               "SROW": np.ascontiguousarray(prep["SROW"][c]),
               "E2": e2(M1)}
              for c in range(C)])
    T1 = _assemble(prep,
                   [np.asarray(r1[c]["T1P"])[0:HID] for c in range(C)],
                   [np.asarray(r1[c]["T1P"])[HID:P] for c in range(C)], F8)

    # ---- L2: T2 = (s*M2)*relu(s*Agg(T1)/M1 + b1) ----
    r2 = run(_prog_agg(meta, 2),
             [{"MSG": _stage_msgs(T1, prep["srcmap"][c]),
               "SROW": np.ascontiguousarray(prep["SROW"][c]),
               "E2": e2(1.0 / M1), "BK": B1K, "ID": IDENT8}
              for c in range(C)])
    T2 = _assemble(prep,
                   [np.asarray(r2[c]["OUT"])[0:HID] for c in range(C)],
                   [np.asarray(r2[c]["OUT"])[HID:P] for c in range(C)], F8)

    # ---- L3: T3 = (s*M3)*(relu((s*Agg(T2)/M2)@W2 + b2)@W3) ----
    r3 = run(_prog_l3(meta),
             [{"MSG": _stage_msgs(T2, prep["srcmap"][c]),
               "SA": np.ascontiguousarray(prep["SA3"][c]),
               "W2": np.concatenate([W2f, W2f], axis=0), "W3": W3f,
               "B2": B2r, "ID": IDENT8}
              for c in range(C)])
    T3 = _assemble(prep,
                   [np.asarray(r3[c]["T3P"])[0:HID] for c in range(C)],
                   [np.asarray(r3[c]["T3P"])[HID:P] for c in range(C)], F8)

    # ---- L4: H3 = s*Agg(T3)/M3 + b3 ----
    r4 = run(_prog_agg(meta, 4),
             [{"MSG": _stage_msgs(T3, prep["srcmap"][c]),
               "SROW": np.ascontiguousarray(prep["SROW"][c]),
               "E2": e2(1.0 / M3), "BK": B3r, "ID": IDENT8}
              for c in range(C)])
    H3 = _assemble(prep,
                   [np.asarray(r4[c]["OUT"])[0:HID] for c in range(C)],
                   [np.asarray(r4[c]["OUT"])[HID:P] for c in range(C)], F16)
    H3[N] = np.float16(-60000.0)     # pad row for the pool staging

    # ---- L5: per-graph max pool ----
    r5 = run(_prog_pool(meta),
             [{"MSGP": _stage_msgs(H3, prep["poolmap"][c])}
              for c in range(C)])
    out = np.empty((N_GRAPHS, HID), np.float32)
    for c in range(C):
        of = np.asarray(r5[c]["OUTF"]).astype(np.float32).T   # ranked graphs
        out[c * GPC + prep["grank"][c]] = of
    out[prep["cnt"].reshape(-1) == 0] = -np.inf
    return out


# revision 47
# speedup vs baseline: 1.0200x; 1.0171x over previous
"""Trainium2 Bass kernel for a 3-layer GCN encoder with global max pool.

Strategy (8 NeuronCores, SPMD, 5 launches, host staging between launches):
  - Nodes partitioned graph-wise (graph g -> core g//64). The host only MOVES
    device-computed bytes between launches (gather rows into padded message
    tables); every FLOP runs on device.
  - Aggregation layers stage per-edge messages in fp8 (e3m4) with fixed
    power-free scale factors folded into the device-side s-tables, halving
    HBM traffic vs fp16.
  - The aggregation sum runs on the Tensor engine: identity-weight matmuls
    accumulate message strips into PSUM (start/stop prefix accumulation).
    Columns (node pairs) are sorted by descending in-degree so the set of
    columns with a message at depth d is a prefix; strips are stored
    region-major (one 512-column PSUM bank region at a time) so PSUM holds
    each region until its sum completes.
  - Post-ops per region: DVE multiplies PSUM by the s-table; Activation
    applies (scaled) bias+relu; DVE writes the next layer's pre-scaled
    fp8 table directly.
  - Launches: L1  T1 = (s*M1) * (X @ W1)                 [fp8 out]
              L2  T2 = (s*M2) * relu(s*Agg(T1)/M1 + b1)  [fp8 out]
              L3  T3 = (s*M3) * (relu((s*Agg(T2)/M2)@W2 + b2)@W3)
              L4  H3 = s*Agg(T3)/M3 + b3                 [fp16 out]
              L5  per-graph max pool over H3 (depth-major staged layout)
"""

import numpy as np
import ml_dtypes

N = 50000
IN_DIM = 128
HID = 64
F2 = 2 * HID
N_GRAPHS = 512
C = 8
P = 128
GPC = N_GRAPHS // C
RW = 512            # psum region width (columns)
F16 = np.float16
F8 = ml_dtypes.float8_e3m4

M1, M2, M3 = 5.0, 11.0, 44.0   # staging scale factors (fold into s-tables)

# aggregation offload: region -> (PE strip count, engine for the rest)
_L3_OFF = {1: (10, "dve"), 2: (12, "pool"), 3: (10, "dve")}
_AGG_OFF = {}


# --------------------------------------------------------------------------
# Host-side preprocessing (graph structure only - no feature arithmetic)
# --------------------------------------------------------------------------

def _host_prep(edge_index, batch):
    src = np.asarray(edge_index[0], dtype=np.int64)
    dst = np.asarray(edge_index[1], dtype=np.int64)
    batch = np.asarray(batch, dtype=np.int64)
    core_of = batch // GPC

    indeg = np.bincount(dst, minlength=N)
    k = indeg + 1                     # slots per node incl. self loop
    s = (1.0 / np.sqrt(k.astype(np.float64))).astype(np.float32)

    # in-neighbor lists grouped by dst
    eorder = np.argsort(dst, kind="stable")
    esrc = src[eorder]
    estart = np.zeros(N + 1, np.int64)
    np.cumsum(np.bincount(dst, minlength=N), out=estart[1:])

    # per-core node order: descending k, paired (2i, 2i+1) into columns
    orders = []
    for c in range(C):
        nodes = np.nonzero(core_of == c)[0]
        orders.append(nodes[np.argsort(-k[nodes], kind="stable")])
    NCOL = max((len(o) + 1) // 2 for o in orders)

    tops = np.full((C, NCOL), -1, np.int64)
    bots = np.full((C, NCOL), -1, np.int64)
    for c in range(C):
        o = orders[c]
        tops[c, : len(o[0::2])] = o[0::2]
        bots[c, : len(o[1::2])] = o[1::2]
    topsx = np.where(tops >= 0, tops, N)
    botsx = np.where(bots >= 0, bots, N)

    # column depth = max over cores of max(k_top, k_bot); >=1 (self loop)
    kk = np.concatenate([k, [0]])
    D_col = np.maximum(kk[topsx], kk[botsx]).max(axis=0)
    D_col = np.maximum(D_col, 1)
    assert (np.diff(D_col) <= 0).all()
    DMAX = int(D_col[0])
    n_d = np.array([(D_col > d).sum() for d in range(DMAX)], np.int64)

    # region widths: full PSUM banks, with the remainder split into a
    # shrinking taper so the final post-op chains are short
    widths = []
    rem = NCOL
    while rem > 704:
        widths.append(RW)
        rem -= RW
    if rem > 384:
        w1 = (rem * 33 // 64) & ~31
        w2 = ((rem - w1) * 3 // 5) & ~31
        widths += [w1, w2, rem - w1 - w2]
    elif rem > 160:
        w1 = (rem * 3 // 5) & ~31
        widths += [w1, rem - w1]
    else:
        widths.append(rem)

    # region-major strips: (region_col0, d, w, slot_off)
    strips = []
    off = 0
    regions = []          # (col0, width, D_r, [strip indices])
    r0 = 0
    for wr in widths:
        Dr = int(D_col[r0])
        idxs = []
        for d in range(Dr):
            w = int(min(n_d[d] - r0, wr))
            assert w > 0
            idxs.append(len(strips))
            strips.append((r0, d, w, off))
            off += w
        regions.append((r0, wr, Dr, idxs))
        r0 += wr
    SLOTS = off

    # slot -> source node maps (N = zero row) for tops/bottoms
    indegx = np.concatenate([indeg, [0]])
    estartx = np.concatenate([estart[:-1], [0]])
    srcmap = np.full((C, 2, SLOTS), N, np.int64)
    for (r0, d, w, soff) in strips:
        for c in range(C):
            for half, nodes_h in ((0, topsx[c]), (1, botsx[c])):
                v = nodes_h[r0 : r0 + w]
                if d == 0:
                    srcmap[c, half, soff : soff + w] = v
                else:
                    sel = (d <= indegx[v]) & (v < N)
                    tgt = srcmap[c, half, soff : soff + w]
                    tgt[sel] = esrc[estartx[v[sel]] + d - 1]

    # s rows [C, 2, NCOL] fp16 (top/bot); the launch-specific scale constant
    # rides in the tiny broadcast matrix E2 instead of a full table.
    sx = np.concatenate([s, [0.0]]).astype(F16)
    SROW = np.stack([sx[topsx], sx[botsx]], axis=1)    # [C, 2, NCOL]

    # L3 keeps a full [128, NCOL] table (PE there is the bottleneck)
    sx3 = np.concatenate([s * (1.0 / M2), [0.0]]).astype(F16)
    top = sx3[topsx][:, None, :].repeat(HID, axis=1)
    bot = sx3[botsx][:, None, :].repeat(HID, axis=1)
    SA3 = np.concatenate([top, bot], axis=1)           # [C, 128, NCOL]

    # pooling: graphs ranked by size per core, split into NPG groups of GH;
    # group G is depth-major: column offG + d*GH + j = d-th node pair of the
    # (G*GH+j)-th largest graph.  Per-group depth S2G trims the rectangle.
    NPG = 4
    GH = GPC // NPG
    gl = batch % GPC
    cnt = np.zeros((C, GPC), np.int64)
    np.add.at(cnt, (core_of, gl), 1)
    grank = np.argsort(-cnt, axis=1, kind="stable")     # [C, GPC] rank->graph
    pairs = -(-cnt // 2)
    S2G = []
    for G in range(NPG):
        S2G.append(int(max(pairs[c, grank[c, G * GH]] for c in range(C))))
    offG = np.zeros(NPG + 1, np.int64)
    np.cumsum(np.array(S2G) * GH, out=offG[1:])
    POOLW = int(offG[-1])
    poolmap = np.full((C, 2, POOLW), N, np.int64)
    for c in range(C):
        for j in range(GPC):
            g = grank[c, j]
            nodes = np.nonzero((core_of == c) & (gl == g))[0]
            e = nodes[0::2]
            o = nodes[1::2]
            G = j // GH
            base = int(offG[G]) + (j % GH)
            poolmap[c, 0, base : base + S2G[G] * GH : GH][: len(e)] = e
            poolmap[c, 1, base : base + S2G[G] * GH : GH][: len(o)] = o

    meta = dict(NCOL=NCOL, SLOTS=SLOTS, strips=strips, regions=regions,
                NPG=NPG, GH=GH, S2G=S2G, offG=[int(x) for x in offG],
                POOLW=POOLW)
    return dict(meta=meta, tops=tops, bots=bots, topsx=topsx, botsx=botsx,
                srcmap=srcmap, poolmap=poolmap, cnt=cnt, grank=grank,
                SROW=SROW, SA3=SA3)


# --------------------------------------------------------------------------
# Bass programs
# --------------------------------------------------------------------------

def _mk_bass():
    import concourse.bacc as bacc
    return bacc.Bacc(None)


def _msg_chunks(strips, SLOTS, first=1400, later=3600):
    """Split the slot axis into DMA chunks at strip boundaries."""
    cuts = []
    target = first
    for (r0, d, w, soff) in strips:
        end = soff + w
        if end >= target:
            cuts.append(end)
            target = end + later
    if not cuts or cuts[-1] != SLOTS:
        cuts.append(SLOTS)
    out = []
    a = 0
    for b in cuts:
        out.append((a, b))
        a = b
    return out


def _prog_agg(meta, layer):
    """L2 (layer==2): OUT = (s*M2)*relu((M1*M2)*(A*s/M1) + (M1*M2)*b1), fp8
       L4 (layer==4): OUT = A*s/M3 + b3, fp16"""
    import concourse.mybir as mybir
    import concourse.tile as tile
    f8 = mybir.dt.float8e3
    f16 = mybir.dt.float16
    f32 = mybir.dt.float32
    Alu = mybir.AluOpType
    Act = mybir.ActivationFunctionType
    NCOL, SLOTS = meta["NCOL"], meta["SLOTS"]
    nc = _mk_bass()

    MSG_d = nc.dram_tensor("MSG", [P, SLOTS], f8, kind="ExternalInput")
    SROW_d = nc.dram_tensor("SROW", [2, NCOL], f16, kind="ExternalInput")
    E2_d = nc.dram_tensor("E2", [2, P], f16, kind="ExternalInput")
    BK_d = nc.dram_tensor("BK", [P, 1], f32, kind="ExternalInput")
    ID_d = nc.dram_tensor("ID", [P, P], f8, kind="ExternalInput")
    OUT_d = nc.dram_tensor("OUT", [P, NCOL], f8 if layer == 2 else f16,
                           kind="ExternalOutput")

    OFF = dict(_AGG_OFF)

    with tile.TileContext(nc, num_cores=C) as tc:
        with (
            tc.tile_pool(name="const", bufs=1) as const,
            tc.tile_pool(name="ps", bufs=3, space="PSUM") as psp,
        ):
            ID_s = const.tile([P, P], f8)
            nc.sync.dma_start(ID_s[:], ID_d[:])
            SROW_s = const.tile([2, NCOL], f16)
            nc.sync.dma_start(SROW_s[:], SROW_d[:])
            E2_s = const.tile([2, P], f16)
            nc.sync.dma_start(E2_s[:], E2_d[:])
            BK_s = const.tile([P, 1], f32)
            nc.sync.dma_start(BK_s[:], BK_d[:])
            MSG_s = const.tile([P, SLOTS], f8)
            chunks = _msg_chunks(meta["strips"], SLOTS)
            for (a, b) in chunks:
                nc.sync.dma_start(MSG_s[:, a:b], MSG_d[:, a:b])
            SA_s = const.tile([P, NCOL], f16)
            U_s = const.tile([P, NCOL], f16)
            H_s = const.tile([P, NCOL], f16)
            OUT_s = const.tile([P, NCOL], f8 if layer == 2 else f16)
            ACC = const.tile([P, NCOL], f16)

            # PE warm-up during the first MSG DMA (pstate ramp); the s-row
            # broadcast matmuls are interleaved into region 0's strip
            # stream below, filling PE's early chunk-wait gaps
            wm = psp.tile([P, RW], f32, tag="ps")
            for i in range(6):
                nc.tensor.matmul(wm[:, :P], lhsT=ID_s[:], rhs=ID_s[:],
                                 start=(i == 0), stop=(i == 5))
            bcast_jobs = list(range(0, NCOL, RW))

            def emit_bcast():
                a = bcast_jobs.pop(0)
                w = min(RW, NCOL - a)
                psSA = psp.tile([P, RW], f32, tag="psb")
                nc.tensor.matmul(psSA[:, :w], lhsT=E2_s[:],
                                 rhs=SROW_s[:, a : a + w],
                                 start=True, stop=True)
                nc.scalar.activation(out=SA_s[:, a : a + w],
                                     in_=psSA[:, :w], func=Act.Copy,
                                     bias=0.0, scale=1.0)

            nreg = len(meta["regions"])
            stored = [0]

            def fin(ri):
                # final OUT mult + store, emitted one region late so the
                # engines pipeline across the last two regions
                (r0, wr, Dr, idxs) = meta["regions"][ri]
                cols = slice(r0, r0 + wr)
                if layer == 2:
                    eng = nc.gpsimd if (ri % 2 == 0 and ri < nreg - 3) \
                        else nc.vector
                    eng.tensor_tensor(out=OUT_s[:, cols], in0=H_s[:, cols],
                                      in1=SA_s[:, cols], op=Alu.mult)
                if ri % 2 == 1 or ri >= nreg - 3 or r0 + wr == NCOL:
                    nc.sync.dma_start(OUT_d[:, stored[0] : r0 + wr],
                                      OUT_s[:, stored[0] : r0 + wr])
                    stored[0] = r0 + wr

            for ri, (r0, wr, Dr, idxs) in enumerate(meta["regions"]):
                split, eng_name = OFF.get(ri, (Dr, None))
                split = min(split, Dr)
                if split < Dr:
                    aeng = nc.vector if eng_name == "dve" else nc.gpsimd
                    first = True
                    for si in idxs[split:]:
                        (_, d, w, soff) = meta["strips"][si]
                        strip = MSG_s[:, soff : soff + w]
                        if first:
                            aeng.tensor_copy(ACC[:, r0 : r0 + w], strip)
                            first = False
                        else:
                            aeng.tensor_tensor(out=ACC[:, r0 : r0 + w],
                                               in0=ACC[:, r0 : r0 + w],
                                               in1=strip, op=Alu.add)
                ps = psp.tile([P, RW], f32, tag="ps")
                for j, si in enumerate(idxs[:split]):
                    (_, d, w, soff) = meta["strips"][si]
                    nc.tensor.matmul(ps[:, :w], lhsT=ID_s[:],
                                     rhs=MSG_s[:, soff : soff + w],
                                     start=(j == 0),
                                     stop=(j == Dr - 1 and split == Dr))
                    if ri <= 1 and j >= 2 and bcast_jobs:
                        emit_bcast()
                while ri == 1 and bcast_jobs:
                    emit_bcast()
                if split < Dr:
                    (_, d, w, soff) = meta["strips"][idxs[split]]
                    nc.tensor.matmul(ps[:, :w], lhsT=ID_s[:],
                                     rhs=ACC[:, r0 : r0 + w],
                                     start=False, stop=True)
                cols = slice(r0, r0 + wr)
                nc.vector.tensor_tensor(out=U_s[:, cols], in0=ps[:, :wr],
                                        in1=SA_s[:, cols], op=Alu.mult)
                if layer == 2:
                    nc.scalar.activation(out=H_s[:, cols], in_=U_s[:, cols],
                                         func=Act.Relu, bias=BK_s[:],
                                         scale=float(M1 * M2))
                else:
                    nc.scalar.activation(out=OUT_s[:, cols], in_=U_s[:, cols],
                                         func=Act.Identity, bias=BK_s[:],
                                         scale=1.0)
                if ri >= 1:
                    fin(ri - 1)
            fin(nreg - 1)
    nc.compile()
    return nc


def _prog_l1(meta):
    """T1 = (s*M1) * (X @ W1), two [64, NCOL] fp8 half strips."""
    import concourse.mybir as mybir
    import concourse.tile as tile
    f8 = mybir.dt.float8e3
    f16 = mybir.dt.float16
    f32 = mybir.dt.float32
    Alu = mybir.AluOpType
    Act = mybir.ActivationFunctionType
    NCOL = meta["NCOL"]
    nc = _mk_bass()

    XT_d = nc.dram_tensor("XT", [IN_DIM, 2 * NCOL], f16, kind="ExternalInput")
    W1_d = nc.dram_tensor("W1", [IN_DIM, HID], f16, kind="ExternalInput")
    SROW_d = nc.dram_tensor("SROW", [2, NCOL], f16, kind="ExternalInput")
    E2_d = nc.dram_tensor("E2", [2, P], f16, kind="ExternalInput")
    T1P_d = nc.dram_tensor("T1P", [P, NCOL], f8, kind="ExternalOutput")

    with tile.TileContext(nc, num_cores=C) as tc:
        with (
            tc.tile_pool(name="const", bufs=1) as const,
            tc.tile_pool(name="ps", bufs=4, space="PSUM") as psp,
        ):
            W1_s = const.tile([IN_DIM, HID], f16)
            nc.sync.dma_start(W1_s[:], W1_d[:])
            SROW_s = const.tile([2, NCOL], f16)
            nc.sync.dma_start(SROW_s[:], SROW_d[:])
            E2_s = const.tile([2, P], f16)
            nc.sync.dma_start(E2_s[:], E2_d[:])
            XT_s = const.tile([IN_DIM, 2 * NCOL], f16)
            SRT1_s = const.tile([P, NCOL], f16)
            cuts = [RW] + list(range(2 * RW, NCOL, 2 * RW)) + [NCOL]
            chunks = []
            a = 0
            for b in cuts:
                if b > a:
                    chunks.append((a, b))
                    a = b
            for ci, (a, b) in enumerate(chunks):
                nc.sync.dma_start(XT_s[:, a:b], XT_d[:, a:b])
                nc.sync.dma_start(XT_s[:, NCOL + a : NCOL + b],
                                  XT_d[:, NCOL + a : NCOL + b])
            T1_s = const.tile([P, NCOL], f8)
            V_s = const.tile([P, NCOL], f16)

            # warm up PE, then broadcast the s-row into SRT1 via PE + Act
            wm = psp.tile([P, RW], f32, tag="ps")
            for i in range(4):
                nc.tensor.matmul(wm[0:HID, :HID], lhsT=W1_s[:], rhs=W1_s[:],
                                 start=(i == 0), stop=(i == 3))
            for a in range(0, NCOL, RW):
                w = min(RW, NCOL - a)
                psSA = psp.tile([P, RW], f32, tag="ps")
                nc.tensor.matmul(psSA[:, :w], lhsT=E2_s[:],
                                 rhs=SROW_s[:, a : a + w],
                                 start=True, stop=True)
                nc.scalar.activation(out=SRT1_s[:, a : a + w],
                                     in_=psSA[:, :w], func=Act.Copy,
                                     bias=0.0, scale=1.0)

            stored = 0
            nflows = -(-NCOL // RW)
            for i in range(nflows):
                a = i * RW
                w = min(RW, NCOL - a)
                ps = psp.tile([P, RW], f32, tag="ps")
                for half in range(2):
                    nc.tensor.matmul(
                        ps[half * HID : half * HID + HID, :w], lhsT=W1_s[:],
                        rhs=XT_s[:, half * NCOL + a : half * NCOL + a + w],
                        start=True, stop=True)
                nc.vector.tensor_tensor(
                    out=T1_s[:, a : a + w], in0=ps[:, :w],
                    in1=SRT1_s[:, a : a + w], op=Alu.mult)
                if i % 2 == 1 or i >= nflows - 2 or a + w == NCOL:
                    nc.sync.dma_start(T1P_d[:, stored : a + w],
                                      T1_s[:, stored : a + w])
                    stored = a + w
    nc.compile()
    return nc


def _prog_l3(meta):
    """T3 = (s*M3)*(relu((s*Agg(T2)/M2)@W2 + b2)@W3), fp8 half strips.
    W3 input is pre-scaled by M2*M3 on the host."""
    import concourse.mybir as mybir
    import concourse.tile as tile
    f8 = mybir.dt.float8e3
    f16 = mybir.dt.float16
    f32 = mybir.dt.float32
    Alu = mybir.AluOpType
    Act = mybir.ActivationFunctionType
    NCOL, SLOTS = meta["NCOL"], meta["SLOTS"]
    nc = _mk_bass()

    MSG_d = nc.dram_tensor("MSG", [P, SLOTS], f8, kind="ExternalInput")
    SA_d = nc.dram_tensor("SA", [P, NCOL], f16, kind="ExternalInput")
    W2_d = nc.dram_tensor("W2", [P, F2], f16, kind="ExternalInput")
    W3_d = nc.dram_tensor("W3", [F2, HID], f16, kind="ExternalInput")
    B2_d = nc.dram_tensor("B2", [F2, 1], f32, kind="ExternalInput")
    ID_d = nc.dram_tensor("ID", [P, P], f8, kind="ExternalInput")
    T3P_d = nc.dram_tensor("T3P", [P, NCOL], f8, kind="ExternalOutput")

    # deep strips of these regions are tree-summed on DVE/Pool into an fp16
    # partial; PE folds the partial into PSUM with one extra pass
    OFF = dict(_L3_OFF)

    with tile.TileContext(nc, num_cores=C) as tc:
        with (
            tc.tile_pool(name="const", bufs=1) as const,
            tc.tile_pool(name="psA", bufs=2, space="PSUM") as psAp,
            tc.tile_pool(name="ps2", bufs=4, space="PSUM") as ps2p,
            tc.tile_pool(name="ps3", bufs=2, space="PSUM") as ps3p,
        ):
            ID_s = const.tile([P, P], f8)
            nc.sync.dma_start(ID_s[:], ID_d[:])
            MSG_s = const.tile([P, SLOTS], f8)
            chunks = _msg_chunks(meta["strips"], SLOTS)
            SA_s = const.tile([P, NCOL], f16)
            W2_s = const.tile([P, F2], f16)
            W3_s = const.tile([F2, HID], f16)
            B2_s = const.tile([F2, 1], f32)
            sa_cut = [0, NCOL // 3, 2 * NCOL // 3, NCOL]
            for ci, (a, b) in enumerate(chunks):
                nc.sync.dma_start(MSG_s[:, a:b], MSG_d[:, a:b])
                if ci == 0:
                    nc.sync.dma_start(W2_s[:], W2_d[:])
                    nc.sync.dma_start(W3_s[:], W3_d[:])
                    nc.sync.dma_start(B2_s[:], B2_d[:])
                elif ci in (1, 3, 5):
                    k = (ci - 1) // 2
                    nc.sync.dma_start(SA_s[:, sa_cut[k] : sa_cut[k + 1]],
                                      SA_d[:, sa_cut[k] : sa_cut[k + 1]])
            U_s = const.tile([P, NCOL], f16)
            H2E = const.tile([F2, NCOL], f16)
            H2O = const.tile([F2, NCOL], f16)
            T3_s = const.tile([P, NCOL], f8)
            ACC = const.tile([P, NCOL], f16)

            # PE warm-up during the first MSG DMA
            wm = psAp.tile([P, RW], f32, tag="psA")
            for i in range(8):
                nc.tensor.matmul(wm[:, :P], lhsT=ID_s[:], rhs=ID_s[:],
                                 start=(i == 0), stop=(i == 7))

            stored = [0]
            nreg = len(meta["regions"])

            def stage_a(ri):
                (r0, wr, Dr, idxs) = meta["regions"][ri]
                ps = ps_of[ri]
                cols = slice(r0, r0 + wr)
                nc.vector.tensor_tensor(out=U_s[:, cols], in0=ps[:, :wr],
                                        in1=SA_s[:, cols], op=Alu.mult)
                ps2s = []
                for hb, tp in ((0, (0, 0)), (HID, (HID, 0))):
                    ps2 = ps2p.tile([F2, RW], f32, tag="ps2")
                    nc.tensor.matmul(ps2[:, :wr],
                                     lhsT=W2_s[hb : hb + HID, :],
                                     rhs=U_s[hb : hb + HID, cols],
                                     start=True, stop=True, tile_position=tp)
                    ps2s.append(ps2)
                ps2_of[ri] = ps2s
                for ps2, H2 in zip(ps2s, (H2E, H2O)):
                    nc.scalar.activation(out=H2[:, cols], in_=ps2[:, :wr],
                                         func=Act.Relu, bias=B2_s[:],
                                         scale=1.0)

            def stage_c(ri):
                (r0, wr, Dr, idxs) = meta["regions"][ri]
                cols = slice(r0, r0 + wr)
                ps3 = ps3p.tile([P, RW], f32, tag="ps3")
                for hb, H2 in ((0, H2E), (HID, H2O)):
                    nc.tensor.matmul(ps3[hb : hb + HID, :wr], lhsT=W3_s[:],
                                     rhs=H2[:, cols], start=True, stop=True)
                nc.vector.tensor_tensor(out=T3_s[:, cols], in0=ps3[:, :wr],
                                        in1=SA_s[:, cols], op=Alu.mult)
                if ri % 2 == 1 or ri >= nreg - 3 or r0 + wr == NCOL:
                    nc.sync.dma_start(T3P_d[:, stored[0] : r0 + wr],
                                      T3_s[:, stored[0] : r0 + wr])
                    stored[0] = r0 + wr

            ps_of = {}
            ps2_of = {}
            for ri, (r0, wr, Dr, idxs) in enumerate(meta["regions"]):
                split, eng_name = OFF.get(ri, (Dr, None))
                split = min(split, Dr)
                if split < Dr:
                    eng = nc.vector if eng_name == "dve" else nc.gpsimd
                    first = True
                    for si in idxs[split:]:
                        (_, d, w, soff) = meta["strips"][si]
                        strip = MSG_s[:, soff : soff + w]
                        if first:
                            eng.tensor_copy(ACC[:, r0 : r0 + w], strip)
                            first = False
                        else:
                            eng.tensor_tensor(out=ACC[:, r0 : r0 + w],
                                              in0=ACC[:, r0 : r0 + w],
                                              in1=strip, op=Alu.add)
                ps = psAp.tile([P, RW], f32, tag="psA")
                ps_of[ri] = ps
                for j, si in enumerate(idxs[:split]):
                    (_, d, w, soff) = meta["strips"][si]
                    nc.tensor.matmul(ps[:, :w], lhsT=ID_s[:],
                                     rhs=MSG_s[:, soff : soff + w],
                                     start=(j == 0),
                                     stop=(j == Dr - 1 and split == Dr))
                if split < Dr:
                    (_, d, w, soff) = meta["strips"][idxs[split]]
                    nc.tensor.matmul(ps[:, :w], lhsT=ID_s[:],
                                     rhs=ACC[:, r0 : r0 + w],
                                     start=False, stop=True)
                if ri >= 1:
                    stage_a(ri - 1)
                if ri >= 2:
                    stage_c(ri - 2)
            stage_a(nreg - 1)
            stage_c(nreg - 2)
            stage_c(nreg - 1)
    nc.compile()
    return nc


def _prog_pool(meta):
    """Per-graph max over NPG depth-major size-ranked groups; OUTF
    [HID, GPC] f32 is features x ranked graphs (host permutes back)."""
    import concourse.mybir as mybir
    import concourse.tile as tile
    f16 = mybir.dt.float16
    f32 = mybir.dt.float32
    Alu = mybir.AluOpType
    NPG, GH = meta["NPG"], meta["GH"]
    S2G, offG, POOLW = meta["S2G"], meta["offG"], meta["POOLW"]
    nc = _mk_bass()

    MSGP_d = nc.dram_tensor("MSGP", [P, POOLW], f16, kind="ExternalInput")
    OUTF_d = nc.dram_tensor("OUTF", [HID, GPC], f32, kind="ExternalOutput")

    with tile.TileContext(nc, num_cores=C) as tc:
        with (
            tc.tile_pool(name="const", bufs=1) as const,
            tc.tile_pool(name="ps", bufs=2, space="PSUM") as psp,
        ):
            mg = const.tile([P, POOLW], f16)
            PM = const.tile([P, GPC], f16)
            OUTF_s = const.tile([HID, GPC], f32)
            order = list(range(NPG - 1, -1, -1))      # smallest group first
            for G in order:
                nc.sync.dma_start(mg[:, offG[G] : offG[G + 1]],
                                  MSGP_d[:, offG[G] : offG[G + 1]])
            for G in order:
                g0 = offG[G]
                eng = nc.vector
                v = mg[:, g0 : offG[G + 1]].rearrange(
                    "p (d g) -> p d g", g=GH)
                cur = S2G[G]
                while cur > 2:
                    h = cur // 2
                    if cur % 2:
                        eng.tensor_tensor(
                            out=v[:, 0, :], in0=v[:, 0, :],
                            in1=v[:, cur - 1, :], op=Alu.max)
                    eng.tensor_tensor(
                        out=v[:, 0:h, :], in0=v[:, 0:h, :],
                        in1=v[:, h : 2 * h, :], op=Alu.max)
                    cur = h
                gsl = slice(G * GH, (G + 1) * GH)
                if cur == 2:
                    eng.tensor_tensor(out=PM[:, gsl], in0=v[:, 0, :],
                                      in1=v[:, 1, :], op=Alu.max)
                else:
                    eng.tensor_copy(PM[:, gsl], v[:, 0, :])
            bot = psp.tile([HID, GPC], f32, tag="bot")
            nc.vector.tensor_copy(bot[:], PM[HID:P, :])
            nc.vector.tensor_tensor(out=OUTF_s[:], in0=PM[0:HID, :],
                                    in1=bot[:], op=Alu.max)
            nc.sync.dma_start(OUTF_d[:], OUTF_s[:])
    nc.compile()
    return nc


# --------------------------------------------------------------------------
# Entry point
# --------------------------------------------------------------------------

_RUN_KWARGS = {}
_EXEC_NS = []
_PROFILE = False


def _stage_msgs(T_full, srcmap_c):
    """[N+1, HID] table + [2, SLOTS] slot->row map -> [128, SLOTS]."""
    top = T_full[srcmap_c[0]].T      # [64, SLOTS]
    bot = T_full[srcmap_c[1]].T
    return np.ascontiguousarray(np.concatenate([top, bot], axis=0))


def _assemble(prep, parts_E, parts_O, dtype):
    """Per-core [64, NCOL] half strips -> full [N+1, HID] table."""
    T_full = np.zeros((N + 1, HID), dtype)
    for c in range(C):
        tops, bots = prep["tops"][c], prep["bots"][c]
        mE, mO = tops >= 0, bots >= 0
        T_full[tops[mE]] = parts_E[c][:, mE].T
        T_full[bots[mO]] = parts_O[c][:, mO].T
    return T_full


def kernel(data, edge_index, batch, W1, b1, W2, b2, W3, b3):
    from concourse.bass_utils import run_bass_kernel_spmd

    data = np.asarray(data, dtype=np.float32)
    edge_index = np.asarray(edge_index, dtype=np.int32)
    batch_np = np.asarray(batch, dtype=np.int32)

    prep = _host_prep(edge_index, batch_np)
    meta = prep["meta"]
    NCOL = meta["NCOL"]

    IDENT8 = np.eye(P, dtype=F8)
    W1f = np.asarray(W1, np.float32).astype(F16)            # [128, 64]
    W2f = np.asarray(W2, np.float32).astype(F16)            # [64, 128]
    W3f = (np.asarray(W3, np.float32) * (M2 * M3)).astype(F16)
    B1K = (np.tile(np.asarray(b1, np.float32), 2) * (M1 * M2))[:, None].copy()
    B2r = np.asarray(b2, np.float32)[:, None].copy()
    B3r = np.tile(np.asarray(b3, np.float32), 2)[:, None].copy()

    Xx = np.concatenate([data, np.zeros((1, IN_DIM), np.float32)], axis=0)
    XT = np.empty((C, IN_DIM, 2 * NCOL), F16)
    for c in range(C):
        XT[c, :, :NCOL] = Xx[prep["topsx"][c]].T
        XT[c, :, NCOL:] = Xx[prep["botsx"][c]].T

    cores = list(range(C))
    del _EXEC_NS[:]

    def run(nc, in_maps):
        if _PROFILE:
            from concourse.timeline_sim import TimelineSim
            _EXEC_NS.append(TimelineSim(nc, require_finite=False).simulate())
        res = run_bass_kernel_spmd(nc, in_maps, cores, **_RUN_KWARGS)
        if res.exec_time_ns is not None:
            _EXEC_NS.append(res.exec_time_ns)
        return res.results

    def e2(const):
        m = np.zeros((2, P), F16)
        m[0, :HID] = const
        m[1, HID:] = const
        return m

    # ---- L1: T1 = (s*M1) * (X @ W1) ----
    r1 = run(_prog_l1(meta),
             [{"XT": np.ascontiguousarray(XT[c]), "W1": W1f,
               "SROW": np.ascontiguousarray(prep["SROW"][c]),
               "E2": e2(M1)}
              for c in range(C)])
    T1 = _assemble(prep,
                   [np.asarray(r1[c]["T1P"])[0:HID] for c in range(C)],
                   [np.asarray(r1[c]["T1P"])[HID:P] for c in range(C)], F8)

    # ---- L2: T2 = (s*M2)*relu(s*Agg(T1)/M1 + b1) ----
    r2 = run(_prog_agg(meta, 2),
             [{"MSG": _stage_msgs(T1, prep["srcmap"][c]),
               "SROW": np.ascontiguousarray(prep["SROW"][c]),
               "E2": e2(1.0 / M1), "BK": B1K, "ID": IDENT8}
              for c in range(C)])
    T2 = _assemble(prep,
                   [np.asarray(r2[c]["OUT"])[0:HID] for c in range(C)],
                   [np.asarray(r2[c]["OUT"])[HID:P] for c in range(C)], F8)

    # ---- L3: T3 = (s*M3)*(relu((s*Agg(T2)/M2)@W2 + b2)@W3) ----
    r3 = run(_prog_l3(meta),
             [{"MSG": _stage_msgs(T2, prep["srcmap"][c]),
               "SA": np.ascontiguousarray(prep["SA3"][c]),
               "W2": np.concatenate([W2f, W2f], axis=0), "W3": W3f,
               "B2": B2r, "ID": IDENT8}
              for c in range(C)])
    T3 = _assemble(prep,
                   [np.asarray(r3[c]["T3P"])[0:HID] for c in range(C)],
                   [np.asarray(r3[c]["T3P"])[HID:P] for c in range(C)], F8)

    # ---- L4: H3 = s*Agg(T3)/M3 + b3 ----
    r4 = run(_prog_agg(meta, 4),
             [{"MSG": _stage_msgs(T3, prep["srcmap"][c]),
               "SROW": np.ascontiguousarray(prep["SROW"][c]),
               "E2": e2(1.0 / M3), "BK": B3r, "ID": IDENT8}
              for c in range(C)])
    H3 = _assemble(prep,
                   [np.asarray(r4[c]["OUT"])[0:HID] for c in range(C)],
                   [np.asarray(r4[c]["OUT"])[HID:P] for c in range(C)], F16)
    H3[N] = np.float16(-60000.0)     # pad row for the pool staging

    # ---- L5: per-graph max pool ----
    r5 = run(_prog_pool(meta),
             [{"MSGP": _stage_msgs(H3, prep["poolmap"][c])}
              for c in range(C)])
    out = np.empty((N_GRAPHS, HID), np.float32)
    for c in range(C):
        of = np.asarray(r5[c]["OUTF"]).astype(np.float32).T   # ranked graphs
        out[c * GPC + prep["grank"][c]] = of
    out[prep["cnt"].reshape(-1) == 0] = -np.inf
    return out
